# revision 43
# baseline (speedup 1.0000x reference)
"""Trainium2 Bass kernel for nn_MultiHeadCrossAttention_47519518163418.

Sharding: 8 cores = (batch b in {0,1}) x (head h in {0..3}); core c: b=c//4, h=c%4.

v2 design (ACT-exp-bound window, PE fillers):
 - q conv: tap-outer over all 8 PSUM banks, weights replicated 4x on the
   output dim so qrep [128,4096] bf16 comes out partition-replicated.
 - QK: bf16, 4-way row-tiled (tile_position (32r,0)) -> 4 score blocks
   [128,512] land in one [128,2048] PSUM tile per (I,g).
 - exp: one N=2048 ACT (Exp, scale=1/sqrt(32), bias=-2.5) writing fp8-e4m3
   directly in DoubleRow-interleaved layout.  Softmax is invariant to the
   exp bias (cancels in the division).
 - PV: fp8 DoubleRow matmuls (256-row contraction, 2 per (I,g)), v in fp8
   with a ones column for the row sums.
 - All silu/sigmoid via tanh (same ACT table set as exp -> zero table
   swaps); rsqrt/recip via int-bit-trick + Newton on DVE (no ACT sqrt).
 - green/v/purple convs run as paced PE "fillers" inside the exp window;
   the AllGather flow (5 mha chunks + in-band green layer sums + purple
   stat gather) follows the previous kernel.
"""

import sys

if "/opt/trn_rl_repo" not in sys.path:
    sys.path.insert(0, "/opt/trn_rl_repo")

import numpy as np
import ml_dtypes

BF16 = ml_dtypes.bfloat16

NUM_HEADS = 4
EPS = 1e-5
D_HEAD = 32
SCALE = float(D_HEAD) ** -0.5
EXP_BIAS = -2.5
N_PX = 16384.0          # pixels per channel of the upsampled image
N_TOT = 128 * 16384.0   # elements per batch for layer stats

_CORES = list(range(8))
_REPLICA_GROUPS = [[0, 1, 2, 3], [4, 5, 6, 7]]

# int-bit-trick constants
RSQRT_MAGIC = float(0x5F3759DF)
RECIP_MAGIC = float(0x7EF127EA)


# ----------------------------------------------------------------------------
# Host-side helpers
# ----------------------------------------------------------------------------

def pos_encoding_pe(c, L, dtype=np.float32):
    half = c // 2
    pos = np.arange(L, dtype=dtype)
    depths = np.arange(half, dtype=dtype) / half
    rates = 1.0 / (10000.0 ** depths)
    ang = pos[:, None] * rates[None, :]
    pe = np.concatenate([np.sin(ang), np.cos(ang)], axis=-1)  # [L, c]
    return pe.T.astype(dtype)  # [c, L]


def reflect_pad(x):
    return np.pad(x, ((0, 0), (1, 1), (1, 1)), mode="reflect")


def edge_pad(x):
    return np.pad(x, ((0, 0), (1, 1), (1, 1)), mode="edge")


_KSET = {(0, 0): [0], (0, 1): [1, 2], (1, 0): [0, 1], (1, 1): [2]}


def collapse_w2(w):
    """w [co, ci, 3, 3] -> W2 [4 (p=2*pr+pc), 2 (dy), 2 (dx), ci, co]."""
    co, ci = w.shape[0], w.shape[1]
    W2 = np.zeros((4, 2, 2, ci, co), dtype=w.dtype)
    for pr in range(2):
        for pc in range(2):
            p = 2 * pr + pc
            for dy in range(2):
                for dx in range(2):
                    acc = np.zeros((co, ci), dtype=np.float64)
                    for ky in _KSET[(pr, dy)]:
                        for kx in _KSET[(pc, dx)]:
                            acc = acc + w[:, :, ky, kx].astype(np.float64)
                    W2[p, dy, dx] = acc.T.astype(w.dtype)
    return W2


def arrange_to_strips(x2d):
    """x [32, 128, 128] -> arranged [128, 4096] phase-major: partition
    32*(2*pr+pc)+c, free r*64+cc for upsampled pixel (2r+pr, 2cc+pc)."""
    t = x2d.reshape(32, 64, 2, 64, 2)          # c, r, pr, cc, pc
    t = t.transpose(2, 4, 0, 1, 3)              # pr, pc, c, r, cc
    return np.ascontiguousarray(t.reshape(128, 4096))


def unarrange_from_strips(arr):
    t = arr.reshape(2, 2, 32, 64, 64)           # pr, pc, c, r, cc
    t = t.transpose(2, 3, 0, 4, 1)              # c, r, pr, cc, pc
    return np.ascontiguousarray(t.reshape(32, 128, 128))


_PE_Y = None
_PE_S = None


_BATCH_CACHE = {}


def _batch_shared(inputs, b):
    """Padded/PE-added tensors shared by the 4 cores of a batch."""
    key = (id(inputs), b)
    if key in _BATCH_CACHE:
        return _BATCH_CACHE[key]
    y = np.asarray(inputs["y"], dtype=np.float32)[b]
    s = np.asarray(inputs["s"], dtype=np.float32)[b]
    ypepad = np.ascontiguousarray(
        reflect_pad((y + _PE_Y).astype(np.float32)).reshape(2, 128, 66, 66)
    ).astype(BF16)
    yreppad = np.ascontiguousarray(edge_pad(y).reshape(2, 128, 66, 66)).astype(BF16)
    spepad = np.ascontiguousarray(reflect_pad((s + _PE_S).astype(np.float32))).astype(BF16)
    _BATCH_CACHE.clear()
    _BATCH_CACHE[key] = (ypepad, yreppad, spepad)
    return _BATCH_CACHE[key]


def prepare_core_inputs(inputs, core):
    global _PE_Y, _PE_S
    if _PE_Y is None:
        _PE_Y = pos_encoding_pe(256, 64 * 64).reshape(256, 64, 64)
        _PE_S = pos_encoding_pe(128, 128 * 128).reshape(128, 128, 128)
    b, h = core // 4, core % 4
    ch = slice(32 * h, 32 * h + 32)
    s = np.asarray(inputs["s"], dtype=np.float32)[b]

    ypepad, yreppad, spepad = _batch_shared(inputs, b)
    sgate = (arrange_to_strips(np.ascontiguousarray(s[ch])) * 0.5).astype(BF16)

    w_blue_y = np.asarray(inputs["w_blue_y"], dtype=np.float32)[ch]
    w_blue_s = np.asarray(inputs["w_blue_s"], dtype=np.float32)[ch]
    w_green = np.asarray(inputs["w_green"], dtype=np.float32)[ch]
    w_purple = np.asarray(inputs["w_purple"], dtype=np.float32)[ch]

    # q conv weights [18 (tap,kt), 128ci, 32co] (col-tiled by tap on device)
    wq = np.zeros((18, 128, 32), dtype=np.float32)
    for t in range(9):
        ky, kx = t // 3, t % 3
        for kt in range(2):
            wq[t * 2 + kt] = w_blue_y[:, 128 * kt : 128 * kt + 128, ky, kx].T
    wv = np.zeros((9, 128, 32), dtype=np.float32)
    for t in range(9):
        ky, kx = t // 3, t % 3
        wv[t] = w_blue_s[:, :, ky, kx].T

    # green/purple: per-phase collapsed weights (no zero padding) for 4-way
    # column-tiled conv: phase p writes psum partitions 32p..32p+32.
    W2g = collapse_w2(w_green)                   # [4, 2, 2, 256, 32]
    wg = np.zeros((32, 128, 32), dtype=np.float32)
    for p in range(4):
        for dy in range(2):
            for dx in range(2):
                for kt in range(2):
                    idx = ((dy * 2 + dx) * 2 + kt) * 4 + p
                    wg[idx] = W2g[p, dy, dx][128 * kt : 128 * kt + 128, :]
    W2p = collapse_w2(w_purple)                  # [4, 2, 2, 128, 32]
    wp = np.zeros((16, 128, 32), dtype=np.float32)
    for p in range(4):
        for dy in range(2):
            for dx in range(2):
                idx = (dy * 2 + dx) * 4 + p
                wp[idx] = W2p[p, dy, dx]

    affg = np.tile(np.stack(
        [np.asarray(inputs["rho_g"], dtype=np.float32).reshape(128)[ch],
         np.asarray(inputs["gamma_g"], dtype=np.float32).reshape(128)[ch],
         np.asarray(inputs["beta_g"], dtype=np.float32).reshape(128)[ch]],
        axis=1), (4, 1))
    affp = np.tile(np.stack(
        [np.asarray(inputs["rho_p"], dtype=np.float32).reshape(128)[ch],
         np.asarray(inputs["gamma_p"], dtype=np.float32).reshape(128)[ch],
         np.asarray(inputs["beta_p"], dtype=np.float32).reshape(128)[ch]],
        axis=1), (4, 1))

    pmat = np.zeros((128, 128), dtype=np.float32)
    for k in range(4):
        for a in range(4):
            pmat[32 * k : 32 * k + 32, 32 * a : 32 * a + 32] = np.eye(32, dtype=np.float32)
    redmats = np.concatenate([pmat, np.ones((128, 128), np.float32)], axis=1)

    return {
        "ypepad": ypepad,
        "redmats": redmats,
        "yreppad": yreppad,
        "spepad": spepad,
        "sgate": sgate,
        "wq": wq.astype(BF16),
        "wv": wv.astype(BF16),
        "wg": wg.astype(BF16),
        "wp": wp.astype(np.float16),
        "affg": np.ascontiguousarray(affg),
        "affp": np.ascontiguousarray(affp),
    }


def assemble_output(per_core_z, per_core_upy):
    out = np.zeros((2, 256, 128, 128), dtype=np.float32)
    for core in range(8):
        b, h = core // 4, core % 4
        z = np.asarray(per_core_z[core]).astype(np.float32)
        u = np.asarray(per_core_upy[core]).astype(np.float32)
        out[b, 32 * h : 32 * h + 32] = unarrange_from_strips(z)
        out[b, 128 + 32 * h : 128 + 32 * h + 32] = unarrange_from_strips(u)
    return out


# ----------------------------------------------------------------------------
# Bass kernel
# ----------------------------------------------------------------------------

def build_bass(no_cc=False):
    import concourse.bass as bass
    import concourse.tile as tile
    from concourse import bacc, mybir

    f32 = mybir.dt.float32
    i32 = mybir.dt.int32
    bf16 = mybir.dt.bfloat16
    fp16 = mybir.dt.float16
    fp8 = mybir.dt.float8e4
    AF = mybir.ActivationFunctionType
    ALU = mybir.AluOpType
    DR = mybir.MatmulPerfMode.DoubleRow

    nc = bacc.Bacc(num_devices=8)

    # ---- I/O ----
    ypepad_d = nc.declare_dram_parameter("ypepad", [2, 128, 66, 66], bf16, isOutput=False)
    yreppad_d = nc.declare_dram_parameter("yreppad", [2, 128, 66, 66], bf16, isOutput=False)
    spepad_d = nc.declare_dram_parameter("spepad", [128, 130, 130], bf16, isOutput=False)
    sgate_d = nc.declare_dram_parameter("sgate", [128, 4096], bf16, isOutput=False)
    wq_d = nc.declare_dram_parameter("wq", [18, 128, 32], bf16, isOutput=False)
    wv_d = nc.declare_dram_parameter("wv", [9, 128, 32], bf16, isOutput=False)
    wg_d = nc.declare_dram_parameter("wg", [32, 128, 32], bf16, isOutput=False)
    wp_d = nc.declare_dram_parameter("wp", [16, 128, 32], fp16, isOutput=False)
    redmats_d = nc.declare_dram_parameter("redmats", [128, 256], f32, isOutput=False)
    affg_d = nc.declare_dram_parameter("affg", [128, 3], f32, isOutput=False)
    affp_d = nc.declare_dram_parameter("affp", [128, 3], f32, isOutput=False)
    zout_d = nc.declare_dram_parameter("zout", [128, 4096], bf16, isOutput=True)
    upyout_d = nc.declare_dram_parameter("upyout", [128, 4096], bf16, isOutput=True)

    # ---- internal DRAM (collective bounce buffers etc.) ----
    AG_BOUNDS = [(0, 1024), (1024, 2048), (2048, 3072),
                 (3072, 3584), (3584, 4096)]
    ccin = [nc.dram_tensor(f"ccin{k}", [33, c1 - c0], fp16)
            for k, (c0, c1) in enumerate(AG_BOUNDS)]
    ccout = [nc.dram_tensor(f"ccout{k}", [132, c1 - c0], fp16)
             for k, (c0, c1) in enumerate(AG_BOUNDS)]
    AG_GREEN = 2  # chunk whose payload carries green layer sums in-band
    psum_b = nc.dram_tensor("psum_b", [128, 2], f32)    # partition-sum bounce
    psum_b2 = nc.dram_tensor("psum_b2", [128, 2], f32)
    cp_out4 = nc.dram_tensor("cp_out4", [512, 2], f32)
    gls_b = nc.dram_tensor("gls_b", [1, 2], f32)        # layer-sum bounce (green)

    import contextlib

    with tile.TileContext(nc) as tc, contextlib.ExitStack() as ctx:
        pers = ctx.enter_context(tc.tile_pool(name="pers", bufs=1))
        small = ctx.enter_context(tc.tile_pool(name="small", bufs=1))

        # ---------------- weights + constants ----------------
        wq_sb = pers.tile([128, 18, 32], bf16, tag="wq")
        nc.sync.dma_start(out=wq_sb, in_=wq_d[:, :, :].rearrange("t p m -> p t m"))
        wv_sb = pers.tile([128, 9, 32], bf16, tag="wv")
        nc.sync.dma_start(out=wv_sb, in_=wv_d[:, :, :].rearrange("t p m -> p t m"))
        wg_sb = pers.tile([128, 32, 32], bf16, tag="wg")
        wp_sb = pers.tile([128, 16, 32], fp16, tag="wp")
        redmats_sb = pers.tile([128, 256], f32, tag="redmats")
        nc.sync.dma_start(out=redmats_sb, in_=redmats_d[:, :])
        affg_sb = small.tile([128, 3], f32, tag="affg")
        nc.sync.dma_start(out=affg_sb, in_=affg_d[:, :])
        affp_sb = small.tile([128, 3], f32, tag="affp")
        nc.sync.dma_start(out=affp_sb, in_=affp_d[:, :])

        def quake_rsqrt(x, p, tag, eps=EPS, steps=3):
            """[p,1] f32 -> rsqrt(x+eps) via 0x5F3759DF bit trick + Newton.
            DVE only; no ACT table involvement."""
            xe = small.tile([p, 1], f32, tag=tag + "xe", name=tag + "xe")
            nc.vector.tensor_scalar_add(xe, x, eps)
            xh = small.tile([p, 1], f32, tag=tag + "xh", name=tag + "xh")
            nc.vector.tensor_scalar_mul(xh, xe, 0.5)
            yi = small.tile([p, 1], i32, tag=tag + "yi", name=tag + "yi")
            nc.vector.tensor_scalar(out=yi, in0=xe.bitcast(i32), scalar1=-0.5,
                                    scalar2=RSQRT_MAGIC, op0=ALU.mult, op1=ALU.add)
            y = yi.bitcast(f32)
            t = small.tile([p, 1], f32, tag=tag + "t", name=tag + "t")
            for _ in range(steps):
                nc.vector.tensor_mul(t, y, y)
                nc.vector.tensor_mul(t, t, xh)
                nc.vector.tensor_scalar(out=t, in0=t, scalar1=-1.0, scalar2=1.5,
                                        op0=ALU.mult, op1=ALU.add)
                nc.vector.tensor_mul(y, y, t)
            return y

        def schraudolph_recip(x, cols, pool, name, steps=2, out_bf=None):
            """[1,cols] f32 (psum ok) -> 1/x via 0x7EF127EA bit trick + Newton.
            If out_bf is given, the last Newton product lands there (bf16)."""
            yi = pool.tile([1, cols], i32, tag="recyi", name=name + "yi")
            nc.vector.tensor_scalar(out=yi, in0=x.bitcast(i32), scalar1=-1.0,
                                    scalar2=RECIP_MAGIC, op0=ALU.mult, op1=ALU.add)
            y = yi.bitcast(f32)
            t = pool.tile([1, cols], f32, tag="rect", name=name + "t")
            for s in range(steps):
                nc.vector.tensor_mul(t, x, y)
                nc.vector.tensor_scalar(out=t, in0=t, scalar1=-1.0, scalar2=2.0,
                                        op0=ALU.mult, op1=ALU.add)
                if s == steps - 1 and out_bf is not None:
                    nc.vector.tensor_mul(out_bf, y, t)
                    return out_bf
                nc.vector.tensor_mul(y, y, t)
            return y

        def part_sums(mv, tag):
            """[128,2] per-partition (mean, biased var over 4096) ->
            [128,2] (sum, sum of squares over the 4096 pixels)."""
            s2 = small.tile([128, 2], f32, tag=tag, name=tag)
            nc.vector.tensor_scalar_mul(s2[:, 0:1], mv[:, 0:1], 4096.0)
            t = small.tile([128, 1], f32, tag=tag + "t", name=tag + "t")
            nc.vector.tensor_mul(t, mv[:, 0:1], mv[:, 0:1])
            nc.vector.tensor_add(t, t, mv[:, 1:2])
            nc.vector.tensor_scalar_mul(s2[:, 1:2], t, 4096.0)
            return s2

        def iln_local(ch_sums, aff_sb, tag, p=128):
            """Per-channel (sum, sumsq) [p,2] -> (t3 = rho*inv_in, u1 = in_m*t3)."""
            n, n1 = N_PX, N_PX - 1.0
            in_m = small.tile([p, 1], f32, tag=tag + "im", name=tag + "im")
            nc.vector.tensor_scalar_mul(in_m, ch_sums[:, 0:1], 1.0 / n)
            t1 = small.tile([p, 1], f32, tag=tag + "t1", name=tag + "t1")
            nc.vector.tensor_mul(t1, ch_sums[:, 0:1], ch_sums[:, 0:1])
            nc.vector.tensor_scalar_mul(t1, t1, 1.0 / n)
            nc.vector.tensor_sub(t1, ch_sums[:, 1:2], t1)
            in_v = small.tile([p, 1], f32, tag=tag + "iv", name=tag + "iv")
            nc.vector.tensor_scalar_mul(in_v, t1, 1.0 / n1)
            inv_in = quake_rsqrt(in_v, p, tag + "ii")
            rho = aff_sb[:, 0:1]
            t3 = small.tile([p, 1], f32, tag=tag + "t3", name=tag + "t3")
            nc.vector.tensor_mul(t3, rho, inv_in)
            u1 = small.tile([p, 1], f32, tag=tag + "u1", name=tag + "u1")
            nc.vector.tensor_mul(u1, in_m, t3)
            return t3, u1

        def iln_post(t3, u1, S_col, aff_sb, tag, p=128):
            """Layer stats half + combine -> [p,2] (scale/2, bias/2) for the
            tanh-based activation path."""
            nt, nt1 = N_TOT, N_TOT - 1.0
            ln_m = small.tile([p, 1], f32, tag=tag + "lm", name=tag + "lm")
            nc.vector.tensor_scalar_mul(ln_m, S_col[:, 0:1], 1.0 / nt)
            l1 = small.tile([p, 1], f32, tag=tag + "l1", name=tag + "l1")
            nc.vector.tensor_mul(l1, S_col[:, 0:1], S_col[:, 0:1])
            nc.vector.tensor_scalar_mul(l1, l1, 1.0 / nt)
            nc.vector.tensor_sub(l1, S_col[:, 1:2], l1)
            ln_v = small.tile([p, 1], f32, tag=tag + "lv", name=tag + "lv")
            nc.vector.tensor_scalar_mul(ln_v, l1, 1.0 / nt1)
            inv_ln = quake_rsqrt(ln_v, p, tag + "il")

            rho = aff_sb[:, 0:1]
            t6 = small.tile([p, 1], f32, tag=tag + "t6", name=tag + "t6")
            nc.vector.tensor_mul(t6, rho, inv_ln)
            nc.vector.tensor_sub(t6, inv_ln, t6)
            A = small.tile([p, 1], f32, tag=tag + "A", name=tag + "A")
            nc.vector.tensor_add(A, t3, t6)
            u2 = small.tile([p, 1], f32, tag=tag + "u2", name=tag + "u2")
            nc.vector.tensor_mul(u2, ln_m, t6)
            nc.vector.tensor_add(u2, u1, u2)
            B = small.tile([p, 1], f32, tag=tag + "B", name=tag + "B")
            nc.vector.tensor_scalar_mul(B, u2, -1.0)
            # sb = (gamma*A/2, (gamma*B + beta)/2) -- halved for tanh path
            sb = small.tile([p, 2], f32, tag=tag + "sb", name=tag + "sb")
            nc.vector.tensor_mul(sb[:, 0:1], A, aff_sb[:, 1:2])
            nc.vector.tensor_scalar_mul(sb[:, 0:1], sb[:, 0:1], 0.5)
            nc.vector.tensor_mul(sb[:, 1:2], B, aff_sb[:, 1:2])
            nc.vector.tensor_add(sb[:, 1:2], sb[:, 1:2], aff_sb[:, 2:3])
            nc.vector.tensor_scalar_mul(sb[:, 1:2], sb[:, 1:2], 0.5)
            return sb

        def layer_sums_local(sums2, bounce_d, tag):
            """sums2 [128,2] -> layer [1,2] via a DRAM bounce + DVE free-dim
            reduce (no PSUM, no gpsimd)."""
            nc.sync.dma_start(out=bounce_d[:, :], in_=sums2)
            tr = small.tile([1, 2, 128], f32, tag=tag + "tr", name=tag + "tr")
            nc.sync.dma_start(
                out=tr, in_=bass.AP(tensor=bounce_d, offset=0,
                                    ap=[[0, 1], [1, 2], [2, 128]]))
            lsb = small.tile([1, 2], f32, tag=tag + "lsb", name=tag + "lsb")
            nc.vector.tensor_reduce(out=lsb.rearrange("p (a b) -> p a b", b=1),
                                    in_=tr, axis=mybir.AxisListType.X, op=ALU.add)
            return lsb

        def group_sum_bcast(tr, bounce_d, tag):
            """tr [1, 2, 4] f32 -> reduce over group axis, bounce, broadcast
            to [128, 2]."""
            tsb = small.tile([1, 2], f32, tag=tag + "tsb", name=tag + "tsb")
            nc.vector.tensor_reduce(out=tsb.rearrange("p (a b) -> p a b", b=1),
                                    in_=tr, axis=mybir.AxisListType.X, op=ALU.add)
            nc.sync.dma_start(out=bounce_d[:, :], in_=tsb)
            bc = small.tile([128, 2], f32, tag=tag + "bc", name=tag + "bc")
            nc.sync.dma_start(
                out=bc, in_=bass.AP(tensor=bounce_d, offset=0, ap=[[0, 128], [1, 2]]))
            return bc

        def emit_body():
            attn = ctx.enter_context(tc.tile_pool(name="attn", bufs=1))
            yrep_pool = ctx.enter_context(tc.tile_pool(name="yrep", bufs=1))

            # ---- big input loads issued first: spe alone on the gpsimd
            # queue (nothing may block ahead of the AG triggers later);
            # everything else on sync after ype.
            spe = yrep_pool.tile([128, 130, 130], bf16, tag="spe")
            for rb in range(5):
                r0, r1 = 26 * rb, 26 * rb + 26
                nc.gpsimd.dma_start(out=spe[:, r0:r1, :], in_=spepad_d[:, r0:r1, :])
            yrep = [yrep_pool.tile([128, 66, 66], bf16, tag=f"yrep{kt}",
                                   name=f"yrep{kt}") for kt in range(2)]
            sgate_sb = attn.tile([128, 4096], bf16, tag="sgate")

            # ================= PE warmup (HAM) =================
            warm_src = pers.tile([128, 512], bf16, tag="warmsrc")
            nc.vector.memset(warm_src[:, :], 0.0)
            with tc.tile_pool(name="warmps", bufs=1, space="PSUM") as wps_pool:
                wps = wps_pool.tile([128, 512], f32, tag="warmps")
                for w in range(24):
                    nc.tensor.matmul(wps[:, :], warm_src[:, 0:128], warm_src[:, :],
                                     start=True, stop=True, skip_group_check=True)

            # ================= q conv (tap-outer, 8 banks) =================
            qstats = small.tile([128, 8, 6], f32, tag="qstats")
            qrep = attn.tile([128, 4096], bf16, tag="qrep")
            with tc.tile_pool(name="inp", bufs=1) as inp, \
                 tc.tile_pool(name="qps", bufs=1, space="PSUM") as qps_pool:
                ype = [inp.tile([128, 66, 66], bf16, tag=f"ype{kt}", name=f"ype{kt}")
                       for kt in range(2)]
                for rb in range(3):
                    for kt in range(2):
                        r0, r1 = 22 * rb, 22 * rb + 22
                        eng = nc.sync if kt == 0 else nc.scalar
                        eng.dma_start(out=ype[kt][:, r0:r1, :], in_=ypepad_d[kt][:, r0:r1, :])
                nc.sync.dma_start(out=wg_sb, in_=wg_d[:, :, :].rearrange("t p m -> p t m"))
                nc.sync.dma_start(out=wp_sb, in_=wp_d[:, :, :].rearrange("t p m -> p t m"))
                for kt in range(2):
                    for rb in range(3):
                        r0, r1 = 22 * rb, 22 * rb + 22
                        nc.sync.dma_start(out=yrep[kt][:, r0:r1, :], in_=yreppad_d[kt][:, r0:r1, :])
                nc.sync.dma_start(out=sgate_sb, in_=sgate_d[:, :])

                # 4-way tap-col-tiled: (tap,kt) idx -> col strip idx%4; strips
                # hold partial sums, reduced on DVE afterwards.
                qps = qps_pool.tile([128, 4096], f32, tag="qps")
                strip_last = [16, 17, 14, 15]   # last idx landing in each strip
                for idx in range(18):
                    t, kt = idx // 2, idx % 2
                    ky, kx = t // 3, t % 3
                    st = idx % 4
                    for c in range(8):
                        r0 = 8 * c
                        nc.tensor.matmul(
                            qps[32 * st : 32 * st + 32, 512 * c : 512 * c + 512],
                            wq_sb[:, idx, :],
                            ype[kt][:, r0 + ky : r0 + ky + 8, kx : kx + 64],
                            start=(idx < 4), stop=(idx == strip_last[st]),
                            tile_position=(0, 32 * st),
                            skip_group_check=True,
                        )
                # strip sums -> qsum [32, 4096] bf16 + IN stats
                # (split across DVE and gpsimd to halve the serial chain)
                qsum = attn.tile([32, 4096], bf16, tag="qsum")
                qtmp = small.tile([32, 512], f32, tag="qtmp")
                qtmp2 = small.tile([32, 512], f32, tag="qtmp2")
                for c in range(8):
                    cols = slice(512 * c, 512 * c + 512)
                    t = qtmp if c % 2 == 0 else qtmp2
                    nc.vector.tensor_copy(t, qps[0:32, cols])
                    nc.vector.tensor_add(t, t, qps[32:64, cols])
                    nc.vector.tensor_add(t, t, qps[64:96, cols])
                    nc.vector.tensor_add(qsum[:, cols], t, qps[96:128, cols])
                    nc.vector.bn_stats(out=qstats[0:32, c, :], in_=qsum[:, cols])
                qmv = small.tile([32, 2], f32, tag="qmv")
                nc.vector.bn_aggr(out=qmv, in_=qstats[0:32])
                qinv = quake_rsqrt(qmv[:, 1:2], 32, "qinv")
                qinv2 = small.tile([32, 1], f32, tag="qinv2")
                nc.vector.tensor_scalar_mul(qinv2, qinv, 0.5)
                qbias2 = small.tile([32, 1], f32, tag="qbias2")
                nc.vector.tensor_scalar(out=qbias2, in0=qmv[:, 0:1], scalar1=qinv2,
                                        scalar2=-1.0, op0=ALU.mult, op1=ALU.mult)
                # silu via tanh: q = xq2 * (1 + tanh(xq2)), xq2 = xhat/2
                qth = attn.tile([32, 4096], bf16, tag="qth32")
                xq2 = attn.tile([32, 4096], bf16, tag="xq232")
                for hh in range(2):
                    cols = slice(2048 * hh, 2048 * hh + 2048)
                    nc.scalar.activation(out=qth[:, cols], in_=qsum[:, cols],
                                         func=AF.Tanh, bias=qbias2, scale=qinv2)
                    nc.vector.tensor_scalar(out=xq2[:, cols], in0=qsum[:, cols],
                                            scalar1=qinv2, scalar2=qbias2,
                                            op0=ALU.mult, op1=ALU.add)
                    nc.vector.scalar_tensor_tensor(
                        out=qrep[0:32, cols], in0=qth[:, cols], scalar=1.0,
                        in1=xq2[:, cols], op0=ALU.add, op1=ALU.mult)
                # replicate q to all four 32-row strips for row-tiled QK
                # via SBUF->SBUF DMA (off the DVE queue)
                for rr in range(1, 4):
                    nc.sync.dma_start(out=qrep[32 * rr : 32 * rr + 32, :],
                                      in_=qrep[0:32, :])

            # ================= attention window =================
            greenraw = attn.tile([128, 4096], bf16, tag="greenraw")
            gstats = small.tile([128, 8, 6], f32, tag="gstats")
            vstats = small.tile([32, 8, 6], f32, tag="vstats")
            vraw = attn.tile([32, 4096], bf16, tag="vraw")
            mhapad = attn.tile([128, 66, 66], fp16, tag="mhapad")
            purpleraw = attn.tile([128, 4096], bf16, tag="purpleraw")
            pstats = small.tile([128, 8, 6], f32, tag="pstats")
            # v tap-outer output [128, 32, 48]: (j%128, jb, ch), ch padded to 48
            vtx = attn.tile([128, 32, 48], bf16, tag="vtx")
            vt8 = attn.tile([128, 32, 48], fp8, tag="vt8")
            vt8_pair = vt8.rearrange("p (pr ko) c -> p pr ko c", ko=2)
            gsums2 = None

            with tc.tile_pool(name="vtmp", bufs=2) as vtmp_pool, \
                 tc.tile_pool(name="stage", bufs=2) as stage_pool, \
                 tc.tile_pool(name="aexpp", bufs=12) as aexp_pool, \
                 tc.tile_pool(name="qkps", bufs=1, space="PSUM") as qkps, \
                 tc.tile_pool(name="pvps", bufs=2, space="PSUM") as pvps, \
                 tc.tile_pool(name="fill", bufs=2, space="PSUM") as fillps:
                expbias = small.tile([128, 1], f32, tag="expbias")
                nc.vector.memset(expbias, EXP_BIAS)
                ones_bf = small.tile([1, 32], bf16, tag="onesbf")
                nc.vector.memset(ones_bf, 1.0)

                # ---------- filler machinery ----------
                fillers = []

                def fill_step(n):
                    for _ in range(n):
                        if fillers:
                            fillers.pop(0)()

                # ---- v conv fillers (4-way tap-col-tiled, 3 rounds/chunk;
                # strips summed on DVE into vraw) ----
                V_GROUPS = [(0, 1), (2, 3), (4, 5), (6, 7)]
                V_STRIP_LAST = [8, 5, 6, 7]     # last tap landing in strip tap%4
                vsum_t = small.tile([32, 512], f32, tag="vsumt")

                def make_v_round(grp, gi, tiles_box, rnd):
                    def run():
                        if rnd == 0:
                            tiles_box.clear()
                            tiles_box.extend(
                                fillps.tile([128, 512], f32, tag="fill",
                                            name=f"vps{gi}_{c}") for c in grp)
                        for tap in range(4 * rnd, min(4 * rnd + 4, 9)):
                            ky, kx = tap // 3, tap % 3
                            st = tap % 4
                            for ci, c in enumerate(grp):
                                r0 = 8 * c
                                nc.tensor.matmul(
                                    tiles_box[ci][32 * st : 32 * st + 32, :],
                                    wv_sb[:, tap, :],
                                    spe[:, 2 * r0 + ky : 2 * r0 + ky + 16 : 2, kx : kx + 128 : 2],
                                    start=(tap < 4), stop=(tap == V_STRIP_LAST[st]),
                                    tile_position=(0, 32 * st),
                                    skip_group_check=True,
                                )
                        if rnd == 2:
                            for ci, c in enumerate(grp):
                                cols = slice(512 * c, 512 * c + 512)
                                t = tiles_box[ci]
                                nc.vector.tensor_copy(vsum_t, t[0:32, :])
                                nc.vector.tensor_add(vsum_t, vsum_t, t[32:64, :])
                                nc.vector.tensor_add(vsum_t, vsum_t, t[64:96, :])
                                nc.vector.tensor_add(vraw[:, cols], vsum_t, t[96:128, :])
                                nc.vector.bn_stats(out=vstats[:, c, :], in_=vraw[:, cols])
                    return run

                vbox = []
                for gi, grp in enumerate(V_GROUPS):
                    for rnd in range(3):
                        fillers.append(make_v_round(grp, gi, vbox, rnd))

                vfin_state = {}

                def v_finalize_half(hh):
                    def run():
                        if hh == 0:
                            vmv = small.tile([32, 2], f32, tag="vmv")
                            nc.vector.bn_aggr(out=vmv, in_=vstats)
                            vinv = quake_rsqrt(vmv[:, 1:2], 32, "vinv")
                            vinv2 = small.tile([32, 1], f32, tag="vinv2")
                            nc.vector.tensor_scalar_mul(vinv2, vinv, 0.5)
                            vbias2 = small.tile([32, 1], f32, tag="vbias2")
                            nc.vector.tensor_scalar(out=vbias2, in0=vmv[:, 0:1],
                                                    scalar1=vinv2, scalar2=-1.0,
                                                    op0=ALU.mult, op1=ALU.mult)
                            vfin_state["xv2"] = vtmp_pool.tile([32, 4096], bf16,
                                                               tag="vtmp", name="xv2")
                            vfin_state["vt32"] = vtmp_pool.tile([32, 4096], bf16,
                                                                tag="vtmp", name="vt32")
                            vfin_state["aff"] = (vinv2, vbias2)
                        xv2 = vfin_state["xv2"]; vt32 = vfin_state["vt32"]
                        vinv2, vbias2 = vfin_state["aff"]
                        vt32_v = vt32.rearrange("p (m c) -> p m c", c=32)
                        cols = slice(2048 * hh, 2048 * hh + 2048)
                        nc.vector.tensor_scalar(out=xv2[:, cols], in0=vraw[:, cols],
                                                scalar1=vinv2, scalar2=vbias2,
                                                op0=ALU.mult, op1=ALU.add)
                        nc.vector.transpose(out=vt32[:, cols], in_=xv2[:, cols])
                        for w in range(4):
                            nc.gpsimd.dma_start(
                                out=vtx[32 * w : 32 * w + 32, 16 * hh : 16 * hh + 16, 0:32],
                                in_=vt32_v[:, 64 * hh + w : 64 * hh + 64 : 4, :])
                        # silu via tanh on the transposed half, cast to fp8
                        hv = vtx.rearrange("p a b -> p (a b)")[:, 768 * hh : 768 * hh + 768]
                        h8 = vt8.rearrange("p a b -> p (a b)")[:, 768 * hh : 768 * hh + 768]
                        tv = vtmp_pool.tile([128, 768], bf16, tag="vth", name=f"tv{hh}")
                        nc.scalar.activation(out=tv, in_=hv, func=AF.Tanh)
                        with nc.allow_low_precision(reason="fp8 v for DoubleRow PV"):
                            nc.vector.scalar_tensor_tensor(
                                out=h8, in0=tv, scalar=1.0, in1=hv,
                                op0=ALU.add, op1=ALU.mult)
                            nc.vector.memset(
                                vt8[:, 16 * hh : 16 * hh + 16, 32:33], 1.0)
                    return run

                fillers.append(v_finalize_half(0))
                fillers.append(v_finalize_half(1))

                # ---- green conv fillers (phase-col-tiled: 8 rounds of 4
                # concurrent phase MMs per chunk; phase p owns partitions
                # 32p..32p+32 of the psum = its 32 output channels) ----
                G_GROUPS = [(0, 1), (2, 3), (4, 5), (6, 7)]

                def make_g_round(grp, gi, tiles_box, rnd):
                    # rnd = (dy*2+dx)*2 + kt in 0..7
                    def run():
                        if rnd == 0:
                            tiles_box.clear()
                            tiles_box.extend(
                                fillps.tile([128, 512], f32, tag="fill",
                                            name=f"gps{gi}_{c}") for c in grp)
                        dy, dx = rnd // 4, (rnd // 2) % 2
                        kt = rnd % 2
                        for p in range(4):
                            pr, pc = p // 2, p % 2
                            ey, ex = pr + dy, pc + dx
                            for ci, c in enumerate(grp):
                                r0 = 8 * c
                                nc.tensor.matmul(
                                    tiles_box[ci][32 * p : 32 * p + 32, :],
                                    wg_sb[:, 4 * rnd + p, :],
                                    yrep[kt][:, r0 + ey : r0 + ey + 8, ex : ex + 64],
                                    start=(rnd == 0), stop=(rnd == 7),
                                    tile_position=(0, 32 * p),
                                    skip_group_check=True,
                                )
                        if rnd == 7:
                            for ci, c in enumerate(grp):
                                cols = slice(512 * c, 512 * c + 512)
                                nc.vector.tensor_copy(greenraw[:, cols], tiles_box[ci][:, :])
                                nc.vector.bn_stats(out=gstats[:, c, :], in_=greenraw[:, cols])
                    return run

                gbox = []
                for gi, grp in enumerate(G_GROUPS):
                    for rnd in range(8):
                        fillers.append(make_g_round(grp, gi, gbox, rnd))

                # ---- purple conv fillers (phase-col-tiled, 4 rounds) ----
                def make_p_round(grp, gi, tiles_box, rnd):
                    # rnd = dy*2+dx in 0..3
                    def run():
                        if rnd == 0:
                            tiles_box.clear()
                            tiles_box.extend(
                                fillps.tile([128, 512], f32, tag="fill",
                                            name=f"pps{gi}_{c}") for c in grp)
                        dy, dx = rnd // 2, rnd % 2
                        for p in range(4):
                            pr, pc = p // 2, p % 2
                            ey, ex = pr + dy, pc + dx
                            for ci, c in enumerate(grp):
                                r0 = 8 * c
                                nc.tensor.matmul(
                                    tiles_box[ci][32 * p : 32 * p + 32, :],
                                    wp_sb[:, 4 * rnd + p, :],
                                    mhapad[:, r0 + ey : r0 + ey + 8, ex : ex + 64],
                                    start=(rnd == 0), stop=(rnd == 3),
                                    tile_position=(0, 32 * p),
                                    skip_group_check=True,
                                )
                        if rnd == 3:
                            for ci, c in enumerate(grp):
                                cols = slice(512 * c, 512 * c + 512)
                                nc.vector.tensor_copy(purpleraw[:, cols], tiles_box[ci][:, :])
                                nc.vector.bn_stats(out=pstats[:, c, :], in_=purpleraw[:, cols])
                    return run

                pbox = []

                def queue_purple(grp, gi):
                    for rnd in range(4):
                        fillers.append(make_p_round(grp, gi, pbox, rnd))

                def emit_divide(k):
                    """Pull AG chunk k's gathered mha rows into mhapad + edge pad.
                    Everything on the gpsimd queue: these ops gate on the
                    AllGather and must not block the DVE/PE/Sync queues."""
                    c0, c1 = AG_BOUNDS[k]
                    w = c1 - c0
                    r0, r1 = c0 // 64, c1 // 64       # image rows
                    nc.gpsimd.dma_start(
                        out=mhapad[:, r0 + 1 : r1 + 1, 1:65],
                        in_=bass.AP(tensor=ccout[k], offset=0,
                                    ap=[[33 * w, 4], [w, 32], [1, w]]))
                    nc.gpsimd.tensor_copy(
                        mhapad[:, r0 + 1 : r1 + 1, 0:1],
                        mhapad[:, r0 + 1 : r1 + 1, 1:2])
                    nc.gpsimd.tensor_copy(
                        mhapad[:, r0 + 1 : r1 + 1, 65:66],
                        mhapad[:, r0 + 1 : r1 + 1, 64:65])
                    if k == 0:
                        nc.gpsimd.tensor_copy(mhapad[:, 0:1, :], mhapad[:, 1:2, :])
                    if k == len(AG_BOUNDS) - 1:
                        nc.gpsimd.tensor_copy(mhapad[:, 65:66, :], mhapad[:, 64:65, :])

                # ---------- the attention I/g loop ----------
                ag_ends = {c1 // 512 - 1: k for k, (c0, c1) in enumerate(AG_BOUNDS)}
                pv_flush = []   # deferred PV work for I=0

                def emit_pv(pvt, g, aexp_t, first, last):
                    """Two fp8 DoubleRow MMs: pairs (2g, 2g+1) into pvt."""
                    for pi in range(2):
                        pair = 2 * g + pi
                        nc.tensor.matmul(
                            pvt[0:33, :],
                            vt8_pair[:, pair, :, 0:33],
                            aexp_t[:, pi, :, :],
                            start=(first and pi == 0), stop=(last and pi == 1),
                            perf_mode=DR,
                            skip_group_check=True,
                        )

                def emit_div_chain(I, pvt):
                    """Reciprocal + broadcast + stage + ccin DMA + AG send for
                    I-block I.  Deferred into the next I's units so the DVE/PE
                    queues never stall on it at an I boundary."""
                    rec = schraudolph_recip(
                        pvt[32:33, :], 512, stage_pool, f"rec{I}",
                        out_bf=stage_pool.tile([1, 512], bf16, tag="recbf",
                                               name=f"recbf{I}"))
                    nc.tensor.matmul(pvt[64:96, :], ones_bf, rec,
                                     start=True, stop=True, skip_group_check=True)
                    rbs = stage_pool.tile([32, 512], f32, tag="rbs", name=f"rbs{I}")
                    nc.vector.tensor_copy(rbs, pvt[64:96, :])
                    stg = stage_pool.tile([32, 512], fp16, tag="stage", name=f"stg{I}")
                    with nc.allow_low_precision(reason="fp16 attention output"):
                        nc.vector.tensor_mul(stg, pvt[0:32, :], rbs)
                    sk = next(k for k, (c0, c1) in enumerate(AG_BOUNDS)
                              if c0 <= 512 * I < c1)
                    lc = 512 * I - AG_BOUNDS[sk][0]
                    nc.sync.dma_start(out=ccin[sk][0:32, lc : lc + 512], in_=stg)

                    if I == 5:
                        # green complete by now: in-band layer sums on chunk 2
                        gmv = small.tile([128, 2], f32, tag="gmv")
                        nc.vector.bn_aggr(out=gmv, in_=gstats)
                        nonlocal gsums2
                        gsums2 = part_sums(gmv, "gsums2")
                        glsb = layer_sums_local(gsums2, psum_b, "gl")
                        nc.sync.dma_start(out=ccin[AG_GREEN][32:33, 0:4],
                                          in_=glsb.bitcast(fp16))
                    if I in ag_ends:
                        k = ag_ends[I]
                        if no_cc:
                            for g4 in range(4):
                                nc.sync.dma_start(
                                    out=ccout[k][33 * g4 : 33 * g4 + 33, :],
                                    in_=ccin[k][:, :])
                        else:
                            nc.gpsimd.collective_compute(
                                "AllGather", ALU.bypass,
                                replica_groups=_REPLICA_GROUPS,
                                ins=[ccin[k][:, :]],
                                outs=[ccout[k][:, :]],
                            )

                pending_div = None
                fill_step(12)   # v conv runs while the q stats/silu chain computes
                for I in range(8):
                    pvt = pvps.tile([128, 512], f32, tag="pvt", name=f"pvt{I}")
                    for g in range(8):
                        if g == 2 and pending_div is not None:
                            emit_div_chain(*pending_div)
                            pending_div = None
                        # QK half-tiles pipeline against the other half's exp:
                        # QK r0/r1 wait only on exp_lo(g-1), r2/r3 on exp_hi(g-1).
                        qk4 = qkps.tile([128, 2048], f32, tag="qk4", name=f"qk4_{I}_{g}")
                        for r in range(4):
                            jb = 4 * g + r
                            nc.tensor.matmul(
                                qk4[:, 512 * r : 512 * r + 512],
                                qrep[32 * r : 32 * r + 32, 128 * jb : 128 * jb + 128],
                                qrep[32 * r : 32 * r + 32, 512 * I : 512 * I + 512],
                                start=True, stop=True,
                                tile_position=(32 * r, 0),
                                skip_group_check=True,
                            )
                        aexp = aexp_pool.tile([128, 2048], fp8, tag="aexp",
                                              name=f"ae{I}_{g}")
                        with nc.allow_low_precision(reason="fp8 attention weights"):
                            nc.scalar.activation(out=aexp[:, 0:1024], in_=qk4[:, 0:1024],
                                                 func=AF.Exp, scale=SCALE, bias=expbias)
                            nc.scalar.activation(out=aexp[:, 1024:2048], in_=qk4[:, 1024:2048],
                                                 func=AF.Exp, scale=SCALE, bias=expbias)
                        aexp_t = aexp.rearrange("p (pr ko i) -> p pr ko i", ko=2, i=512)
                        pv_flush.append((pvt, g, aexp_t, g == 0, g == 7))
                        if I > 0 or g == 7:
                            # steady state: emit PV one unit late so it never
                            # stalls the PE queue head on a fresh exp
                            while len(pv_flush) > (1 if g < 7 else 0):
                                args = pv_flush.pop(0)
                                emit_pv(args[0], args[1], args[2], args[3], args[4])
                        fill_step(1 if I > 0 else 2)

                    pending_div = (I, pvt)
                    # purple fillers as AG chunks land
                    if I == 5:
                        emit_divide(0)
                        queue_purple((0,), 0)
                    if I == 6:
                        emit_divide(1)
                        queue_purple((1, 2), 1)
                        queue_purple((3,), 2)

                emit_div_chain(*pending_div)
                pending_div = None

                # ---- green finalize (gates only on AG2 + gsums2; overlaps
                # the tail AGs) ----
                gx = small.tile([1, 16], fp16, tag="gx")
                w1 = AG_BOUNDS[AG_GREEN][1] - AG_BOUNDS[AG_GREEN][0]
                nc.gpsimd.dma_start(
                    out=gx,
                    in_=bass.AP(tensor=ccout[AG_GREEN], offset=32 * w1,
                                ap=[[2, 2], [33 * w1, 4], [1, 2]]))
                glsum = group_sum_bcast(
                    gx.bitcast(f32).rearrange("p (k g) -> p k g", g=4), gls_b, "gl")
                gfill = fillps.tile([128, 512], f32, tag="fill", name="gch_ps")
                nc.tensor.matmul(gfill[:, 0:2], redmats_sb[:, 0:128], gsums2,
                                 start=True, stop=True, skip_group_check=True)
                gch = small.tile([128, 2], f32, tag="gach", name="gach")
                nc.vector.tensor_copy(gch, gfill[:, 0:2])
                gt3, gu1 = iln_local(gch, affg_sb, "ga")
                gsb = iln_post(gt3, gu1, glsum, affg_sb, "ga")
                # upy = silu(A*x+B) = u2*(1+tanh(u2)), u2 = x*A/2 + B/2
                # (reuse the q-phase slabs: those tiles are dead by now)
                gth = attn.tile([128, 4096], bf16, tag="qth32", name="gth")
                gu2 = attn.tile([128, 4096], bf16, tag="xq232", name="gu2")
                upy_sb = attn.tile([128, 4096], bf16, tag="qsum", name="upy")
                for hh in range(2):
                    cols = slice(2048 * hh, 2048 * hh + 2048)
                    nc.scalar.activation(out=gth[:, cols], in_=greenraw[:, cols],
                                         func=AF.Tanh, bias=gsb[:, 1:2], scale=gsb[:, 0:1])
                    nc.vector.tensor_scalar(out=gu2[:, cols], in0=greenraw[:, cols],
                                            scalar1=gsb[:, 0:1], scalar2=gsb[:, 1:2],
                                            op0=ALU.mult, op1=ALU.add)
                    nc.vector.scalar_tensor_tensor(
                        out=upy_sb[:, cols], in0=gth[:, cols], scalar=1.0,
                        in1=gu2[:, cols], op0=ALU.add, op1=ALU.mult)
                    nc.sync.dma_start(out=upyout_d[:, cols], in_=upy_sb[:, cols])

                emit_divide(2)
                queue_purple((4, 5), 3)
                fill_step(len(fillers))
                emit_divide(3)
                queue_purple((6,), 4)
                emit_divide(4)
                queue_purple((7,), 5)
                fill_step(len(fillers))

                # ---- purple stats + AG + ILN ----
                pmv = small.tile([128, 2], f32, tag="pmv")
                nc.vector.bn_aggr(out=pmv, in_=pstats)
                psums2 = part_sums(pmv, "psums2")
                nc.sync.dma_start(out=psum_b2[:, :], in_=psums2)
                if no_cc:
                    for g4 in range(4):
                        nc.sync.dma_start(out=cp_out4[128 * g4 : 128 * g4 + 128, :],
                                          in_=psum_b2[:, :])
                else:
                    nc.gpsimd.collective_compute(
                        "AllGather", ALU.bypass,
                        replica_groups=_REPLICA_GROUPS,
                        ins=[bass.AP(tensor=psum_b2, offset=0, ap=[[2, 128], [1, 2]])],
                        outs=[bass.AP(tensor=cp_out4, offset=0, ap=[[2, 512], [1, 2]])],
                    )
                pfill = fillps.tile([128, 512], f32, tag="fill", name="pch_ps")
                nc.tensor.matmul(pfill[:, 0:2], redmats_sb[:, 0:128], psums2,
                                 start=True, stop=True, skip_group_check=True)
                pch = small.tile([128, 2], f32, tag="pach", name="pach")
                nc.vector.tensor_copy(pch, pfill[:, 0:2])
                pt3, pu1 = iln_local(pch, affp_sb, "pa")
                ptr = small.tile([128, 2, 4], f32, tag="ptr")
                nc.sync.dma_start(
                    out=ptr, in_=bass.AP(tensor=cp_out4, offset=0,
                                         ap=[[2, 128], [1, 2], [256, 4]]))
                pfill2 = fillps.tile([128, 512], f32, tag="fill", name="pls_ps")
                nc.tensor.matmul(pfill2[:, 0:8],
                                 redmats_sb[:, 128:256],
                                 ptr.rearrange("p a b -> p (a b)"),
                                 start=True, stop=True, skip_group_check=True)
                pls4 = small.tile([128, 2, 4], f32, tag="pls4", name="pls4")
                nc.vector.tensor_copy(pls4.rearrange("p a b -> p (a b)"), pfill2[:, 0:8])
                plsum = small.tile([128, 2], f32, tag="plsum")
                nc.vector.tensor_reduce(out=plsum.rearrange("p (a b) -> p a b", b=1),
                                        in_=pls4, axis=mybir.AxisListType.X, op=ALU.add)
                psb = iln_post(pt3, pu1, plsum, affp_sb, "pa")

                # z = sigmoid(A*x+B)*s = (1+tanh(u2))*(s/2), u2 = x*A/2+B/2
                pth = attn.tile([128, 4096], bf16, tag="qth32", name="pth")
                zpre = attn.tile([128, 4096], bf16, tag="xq232", name="zpre")
                for hh in range(4):
                    cols = slice(1024 * hh, 1024 * hh + 1024)
                    nc.scalar.activation(out=pth[:, cols], in_=purpleraw[:, cols],
                                         func=AF.Tanh, bias=psb[:, 1:2], scale=psb[:, 0:1])
                    nc.vector.scalar_tensor_tensor(
                        out=zpre[:, cols], in0=pth[:, cols], scalar=1.0,
                        in1=sgate_sb[:, cols], op0=ALU.add, op1=ALU.mult)
                    nc.sync.dma_start(out=zout_d[:, cols], in_=zpre[:, cols])

        emit_body()

    nc.compile()
    return nc


_NC_CACHE = None
RUN_KWARGS = {}      # test harness may set e.g. {"trace": True}
LAST_RESULTS = None  # BassKernelResults of the most recent run


def kernel(**inputs) -> np.ndarray:
    global _NC_CACHE, LAST_RESULTS
    from concourse.bass_utils import run_bass_kernel_spmd

    if _NC_CACHE is None:
        _NC_CACHE = build_bass()
    nc = _NC_CACHE

    in_maps = []
    for core in _CORES:
        ci = prepare_core_inputs(inputs, core)
        in_maps.append(ci)

    res = run_bass_kernel_spmd(nc, in_maps, _CORES, **RUN_KWARGS)
    LAST_RESULTS = res
    zs = [res.results[c]["zout"] for c in _CORES]
    upys = [res.results[c]["upyout"] for c in _CORES]
    return assemble_output(zs, upys)


if __name__ == "__main__":
    nc = build_bass()
    print("bass build OK")


# revision 44
# speedup vs baseline: 1.0292x; 1.0292x over previous
"""Trainium2 Bass kernel for nn_MultiHeadCrossAttention_47519518163418.

Sharding: 8 cores = (batch b in {0,1}) x (head h in {0..3}); core c: b=c//4, h=c%4.

v2 design (ACT-exp-bound window, PE fillers):
 - q conv: tap-outer over all 8 PSUM banks, weights replicated 4x on the
   output dim so qrep [128,4096] bf16 comes out partition-replicated.
 - QK: bf16, 4-way row-tiled (tile_position (32r,0)) -> 4 score blocks
   [128,512] land in one [128,2048] PSUM tile per (I,g).
 - exp: one N=2048 ACT (Exp, scale=1/sqrt(32), bias=-2.5) writing fp8-e4m3
   directly in DoubleRow-interleaved layout.  Softmax is invariant to the
   exp bias (cancels in the division).
 - PV: fp8 DoubleRow matmuls (256-row contraction, 2 per (I,g)), v in fp8
   with a ones column for the row sums.
 - All silu/sigmoid via tanh (same ACT table set as exp -> zero table
   swaps); rsqrt/recip via int-bit-trick + Newton on DVE (no ACT sqrt).
 - green/v/purple convs run as paced PE "fillers" inside the exp window;
   the AllGather flow (5 mha chunks + in-band green layer sums + purple
   stat gather) follows the previous kernel.
"""

import sys

if "/opt/trn_rl_repo" not in sys.path:
    sys.path.insert(0, "/opt/trn_rl_repo")

import numpy as np
import ml_dtypes

BF16 = ml_dtypes.bfloat16

NUM_HEADS = 4
EPS = 1e-5
D_HEAD = 32
SCALE = float(D_HEAD) ** -0.5
EXP_BIAS = -2.5
N_PX = 16384.0          # pixels per channel of the upsampled image
N_TOT = 128 * 16384.0   # elements per batch for layer stats

_CORES = list(range(8))
_REPLICA_GROUPS = [[0, 1, 2, 3], [4, 5, 6, 7]]

# int-bit-trick constants
RSQRT_MAGIC = float(0x5F3759DF)
RECIP_MAGIC = float(0x7EF127EA)


# ----------------------------------------------------------------------------
# Host-side helpers
# ----------------------------------------------------------------------------

def pos_encoding_pe(c, L, dtype=np.float32):
    half = c // 2
    pos = np.arange(L, dtype=dtype)
    depths = np.arange(half, dtype=dtype) / half
    rates = 1.0 / (10000.0 ** depths)
    ang = pos[:, None] * rates[None, :]
    pe = np.concatenate([np.sin(ang), np.cos(ang)], axis=-1)  # [L, c]
    return pe.T.astype(dtype)  # [c, L]


def reflect_pad(x):
    return np.pad(x, ((0, 0), (1, 1), (1, 1)), mode="reflect")


def edge_pad(x):
    return np.pad(x, ((0, 0), (1, 1), (1, 1)), mode="edge")


_KSET = {(0, 0): [0], (0, 1): [1, 2], (1, 0): [0, 1], (1, 1): [2]}


def collapse_w2(w):
    """w [co, ci, 3, 3] -> W2 [4 (p=2*pr+pc), 2 (dy), 2 (dx), ci, co]."""
    co, ci = w.shape[0], w.shape[1]
    W2 = np.zeros((4, 2, 2, ci, co), dtype=w.dtype)
    for pr in range(2):
        for pc in range(2):
            p = 2 * pr + pc
            for dy in range(2):
                for dx in range(2):
                    acc = np.zeros((co, ci), dtype=np.float64)
                    for ky in _KSET[(pr, dy)]:
                        for kx in _KSET[(pc, dx)]:
                            acc = acc + w[:, :, ky, kx].astype(np.float64)
                    W2[p, dy, dx] = acc.T.astype(w.dtype)
    return W2


def arrange_to_strips(x2d):
    """x [32, 128, 128] -> arranged [128, 4096] phase-major: partition
    32*(2*pr+pc)+c, free r*64+cc for upsampled pixel (2r+pr, 2cc+pc)."""
    t = x2d.reshape(32, 64, 2, 64, 2)          # c, r, pr, cc, pc
    t = t.transpose(2, 4, 0, 1, 3)              # pr, pc, c, r, cc
    return np.ascontiguousarray(t.reshape(128, 4096))


def unarrange_from_strips(arr):
    t = arr.reshape(2, 2, 32, 64, 64)           # pr, pc, c, r, cc
    t = t.transpose(2, 3, 0, 4, 1)              # c, r, pr, cc, pc
    return np.ascontiguousarray(t.reshape(32, 128, 128))


_PE_Y = None
_PE_S = None


_BATCH_CACHE = {}


def _batch_shared(inputs, b):
    """Padded/PE-added tensors shared by the 4 cores of a batch."""
    key = (id(inputs), b)
    if key in _BATCH_CACHE:
        return _BATCH_CACHE[key]
    y = np.asarray(inputs["y"], dtype=np.float32)[b]
    s = np.asarray(inputs["s"], dtype=np.float32)[b]
    ypepad = np.ascontiguousarray(
        reflect_pad((y + _PE_Y).astype(np.float32)).reshape(2, 128, 66, 66)
    ).astype(BF16)
    yreppad = np.ascontiguousarray(edge_pad(y).reshape(2, 128, 66, 66)).astype(BF16)
    spepad = np.ascontiguousarray(reflect_pad((s + _PE_S).astype(np.float32))).astype(BF16)
    _BATCH_CACHE.clear()
    _BATCH_CACHE[key] = (ypepad, yreppad, spepad)
    return _BATCH_CACHE[key]


def prepare_core_inputs(inputs, core):
    global _PE_Y, _PE_S
    if _PE_Y is None:
        _PE_Y = pos_encoding_pe(256, 64 * 64).reshape(256, 64, 64)
        _PE_S = pos_encoding_pe(128, 128 * 128).reshape(128, 128, 128)
    b, h = core // 4, core % 4
    ch = slice(32 * h, 32 * h + 32)
    s = np.asarray(inputs["s"], dtype=np.float32)[b]

    ypepad, yreppad, spepad = _batch_shared(inputs, b)
    sgate = (arrange_to_strips(np.ascontiguousarray(s[ch])) * 0.5).astype(BF16)

    w_blue_y = np.asarray(inputs["w_blue_y"], dtype=np.float32)[ch]
    w_blue_s = np.asarray(inputs["w_blue_s"], dtype=np.float32)[ch]
    w_green = np.asarray(inputs["w_green"], dtype=np.float32)[ch]
    w_purple = np.asarray(inputs["w_purple"], dtype=np.float32)[ch]

    # q conv weights [18 (tap,kt), 128ci, 32co] (col-tiled by tap on device)
    wq = np.zeros((18, 128, 32), dtype=np.float32)
    for t in range(9):
        ky, kx = t // 3, t % 3
        for kt in range(2):
            wq[t * 2 + kt] = w_blue_y[:, 128 * kt : 128 * kt + 128, ky, kx].T
    wv = np.zeros((9, 128, 32), dtype=np.float32)
    for t in range(9):
        ky, kx = t // 3, t % 3
        wv[t] = w_blue_s[:, :, ky, kx].T

    # green/purple: per-phase collapsed weights (no zero padding) for 4-way
    # column-tiled conv: phase p writes psum partitions 32p..32p+32.
    W2g = collapse_w2(w_green)                   # [4, 2, 2, 256, 32]
    wg = np.zeros((32, 128, 32), dtype=np.float32)
    for p in range(4):
        for dy in range(2):
            for dx in range(2):
                for kt in range(2):
                    idx = ((dy * 2 + dx) * 2 + kt) * 4 + p
                    wg[idx] = W2g[p, dy, dx][128 * kt : 128 * kt + 128, :]
    W2p = collapse_w2(w_purple)                  # [4, 2, 2, 128, 32]
    wp = np.zeros((16, 128, 32), dtype=np.float32)
    for p in range(4):
        for dy in range(2):
            for dx in range(2):
                idx = (dy * 2 + dx) * 4 + p
                wp[idx] = W2p[p, dy, dx]

    affg = np.tile(np.stack(
        [np.asarray(inputs["rho_g"], dtype=np.float32).reshape(128)[ch],
         np.asarray(inputs["gamma_g"], dtype=np.float32).reshape(128)[ch],
         np.asarray(inputs["beta_g"], dtype=np.float32).reshape(128)[ch]],
        axis=1), (4, 1))
    affp = np.tile(np.stack(
        [np.asarray(inputs["rho_p"], dtype=np.float32).reshape(128)[ch],
         np.asarray(inputs["gamma_p"], dtype=np.float32).reshape(128)[ch],
         np.asarray(inputs["beta_p"], dtype=np.float32).reshape(128)[ch]],
        axis=1), (4, 1))

    pmat = np.zeros((128, 128), dtype=np.float32)
    for k in range(4):
        for a in range(4):
            pmat[32 * k : 32 * k + 32, 32 * a : 32 * a + 32] = np.eye(32, dtype=np.float32)
    redmats = np.concatenate([pmat, np.ones((128, 128), np.float32)], axis=1)

    return {
        "ypepad": ypepad,
        "redmats": redmats,
        "yreppad": yreppad,
        "spepad": spepad,
        "sgate": sgate,
        "wq": wq.astype(BF16),
        "wv": wv.astype(BF16),
        "wg": wg.astype(BF16),
        "wp": wp.astype(np.float16),
        "affg": np.ascontiguousarray(affg),
        "affp": np.ascontiguousarray(affp),
    }


def assemble_output(per_core_z, per_core_upy):
    out = np.zeros((2, 256, 128, 128), dtype=np.float32)
    for core in range(8):
        b, h = core // 4, core % 4
        z = np.asarray(per_core_z[core]).astype(np.float32)
        u = np.asarray(per_core_upy[core]).astype(np.float32)
        out[b, 32 * h : 32 * h + 32] = unarrange_from_strips(z)
        out[b, 128 + 32 * h : 128 + 32 * h + 32] = unarrange_from_strips(u)
    return out


# ----------------------------------------------------------------------------
# Bass kernel
# ----------------------------------------------------------------------------

def build_bass(no_cc=False):
    import concourse.bass as bass
    import concourse.tile as tile
    from concourse import bacc, mybir

    f32 = mybir.dt.float32
    i32 = mybir.dt.int32
    bf16 = mybir.dt.bfloat16
    fp16 = mybir.dt.float16
    fp8 = mybir.dt.float8e4
    AF = mybir.ActivationFunctionType
    ALU = mybir.AluOpType
    DR = mybir.MatmulPerfMode.DoubleRow

    nc = bacc.Bacc(num_devices=8)

    # ---- I/O ----
    ypepad_d = nc.declare_dram_parameter("ypepad", [2, 128, 66, 66], bf16, isOutput=False)
    yreppad_d = nc.declare_dram_parameter("yreppad", [2, 128, 66, 66], bf16, isOutput=False)
    spepad_d = nc.declare_dram_parameter("spepad", [128, 130, 130], bf16, isOutput=False)
    sgate_d = nc.declare_dram_parameter("sgate", [128, 4096], bf16, isOutput=False)
    wq_d = nc.declare_dram_parameter("wq", [18, 128, 32], bf16, isOutput=False)
    wv_d = nc.declare_dram_parameter("wv", [9, 128, 32], bf16, isOutput=False)
    wg_d = nc.declare_dram_parameter("wg", [32, 128, 32], bf16, isOutput=False)
    wp_d = nc.declare_dram_parameter("wp", [16, 128, 32], fp16, isOutput=False)
    redmats_d = nc.declare_dram_parameter("redmats", [128, 256], f32, isOutput=False)
    affg_d = nc.declare_dram_parameter("affg", [128, 3], f32, isOutput=False)
    affp_d = nc.declare_dram_parameter("affp", [128, 3], f32, isOutput=False)
    zout_d = nc.declare_dram_parameter("zout", [128, 4096], bf16, isOutput=True)
    upyout_d = nc.declare_dram_parameter("upyout", [128, 4096], bf16, isOutput=True)

    # ---- internal DRAM (collective bounce buffers etc.) ----
    AG_BOUNDS = [(0, 512), (512, 1536), (1536, 2560),
                 (2560, 3584), (3584, 4096)]
    ccin = [nc.dram_tensor(f"ccin{k}", [33, c1 - c0], fp16)
            for k, (c0, c1) in enumerate(AG_BOUNDS)]
    ccout = [nc.dram_tensor(f"ccout{k}", [132, c1 - c0], fp16)
             for k, (c0, c1) in enumerate(AG_BOUNDS)]
    AG_GREEN = 3  # chunk whose payload carries green layer sums in-band
    psum_b = nc.dram_tensor("psum_b", [128, 2], f32)    # partition-sum bounce
    psum_b2 = nc.dram_tensor("psum_b2", [128, 2], f32)
    cp_out4 = nc.dram_tensor("cp_out4", [512, 2], f32)
    gls_b = nc.dram_tensor("gls_b", [1, 2], f32)        # layer-sum bounce (green)

    import contextlib

    with tile.TileContext(nc) as tc, contextlib.ExitStack() as ctx:
        pers = ctx.enter_context(tc.tile_pool(name="pers", bufs=1))
        small = ctx.enter_context(tc.tile_pool(name="small", bufs=1))

        # ---------------- weights + constants ----------------
        wq_sb = pers.tile([128, 18, 32], bf16, tag="wq")
        nc.sync.dma_start(out=wq_sb, in_=wq_d[:, :, :].rearrange("t p m -> p t m"))
        wv_sb = pers.tile([128, 9, 32], bf16, tag="wv")
        nc.sync.dma_start(out=wv_sb, in_=wv_d[:, :, :].rearrange("t p m -> p t m"))
        wg_sb = pers.tile([128, 32, 32], bf16, tag="wg")
        wp_sb = pers.tile([128, 16, 32], fp16, tag="wp")
        redmats_sb = pers.tile([128, 256], f32, tag="redmats")
        nc.sync.dma_start(out=redmats_sb, in_=redmats_d[:, :])
        affg_sb = small.tile([128, 3], f32, tag="affg")
        nc.sync.dma_start(out=affg_sb, in_=affg_d[:, :])
        affp_sb = small.tile([128, 3], f32, tag="affp")
        nc.sync.dma_start(out=affp_sb, in_=affp_d[:, :])

        def quake_rsqrt(x, p, tag, eps=EPS, steps=3):
            """[p,1] f32 -> rsqrt(x+eps) via 0x5F3759DF bit trick + Newton.
            DVE only; no ACT table involvement."""
            xe = small.tile([p, 1], f32, tag=tag + "xe", name=tag + "xe")
            nc.vector.tensor_scalar_add(xe, x, eps)
            xh = small.tile([p, 1], f32, tag=tag + "xh", name=tag + "xh")
            nc.vector.tensor_scalar_mul(xh, xe, 0.5)
            yi = small.tile([p, 1], i32, tag=tag + "yi", name=tag + "yi")
            nc.vector.tensor_scalar(out=yi, in0=xe.bitcast(i32), scalar1=-0.5,
                                    scalar2=RSQRT_MAGIC, op0=ALU.mult, op1=ALU.add)
            y = yi.bitcast(f32)
            t = small.tile([p, 1], f32, tag=tag + "t", name=tag + "t")
            for _ in range(steps):
                nc.vector.tensor_mul(t, y, y)
                nc.vector.tensor_mul(t, t, xh)
                nc.vector.tensor_scalar(out=t, in0=t, scalar1=-1.0, scalar2=1.5,
                                        op0=ALU.mult, op1=ALU.add)
                nc.vector.tensor_mul(y, y, t)
            return y

        def schraudolph_recip(x, cols, pool, name, steps=2, out_bf=None):
            """[1,cols] f32 (psum ok) -> 1/x via 0x7EF127EA bit trick + Newton.
            If out_bf is given, the last Newton product lands there (bf16)."""
            yi = pool.tile([1, cols], i32, tag="recyi", name=name + "yi")
            nc.vector.tensor_scalar(out=yi, in0=x.bitcast(i32), scalar1=-1.0,
                                    scalar2=RECIP_MAGIC, op0=ALU.mult, op1=ALU.add)
            y = yi.bitcast(f32)
            t = pool.tile([1, cols], f32, tag="rect", name=name + "t")
            for s in range(steps):
                nc.vector.tensor_mul(t, x, y)
                nc.vector.tensor_scalar(out=t, in0=t, scalar1=-1.0, scalar2=2.0,
                                        op0=ALU.mult, op1=ALU.add)
                if s == steps - 1 and out_bf is not None:
                    nc.vector.tensor_mul(out_bf, y, t)
                    return out_bf
                nc.vector.tensor_mul(y, y, t)
            return y

        def part_sums(mv, tag):
            """[128,2] per-partition (mean, biased var over 4096) ->
            [128,2] (sum, sum of squares over the 4096 pixels)."""
            s2 = small.tile([128, 2], f32, tag=tag, name=tag)
            nc.vector.tensor_scalar_mul(s2[:, 0:1], mv[:, 0:1], 4096.0)
            t = small.tile([128, 1], f32, tag=tag + "t", name=tag + "t")
            nc.vector.tensor_mul(t, mv[:, 0:1], mv[:, 0:1])
            nc.vector.tensor_add(t, t, mv[:, 1:2])
            nc.vector.tensor_scalar_mul(s2[:, 1:2], t, 4096.0)
            return s2

        def iln_local(ch_sums, aff_sb, tag, p=128):
            """Per-channel (sum, sumsq) [p,2] -> (t3 = rho*inv_in, u1 = in_m*t3)."""
            n, n1 = N_PX, N_PX - 1.0
            in_m = small.tile([p, 1], f32, tag=tag + "im", name=tag + "im")
            nc.vector.tensor_scalar_mul(in_m, ch_sums[:, 0:1], 1.0 / n)
            t1 = small.tile([p, 1], f32, tag=tag + "t1", name=tag + "t1")
            nc.vector.tensor_mul(t1, ch_sums[:, 0:1], ch_sums[:, 0:1])
            nc.vector.tensor_scalar_mul(t1, t1, 1.0 / n)
            nc.vector.tensor_sub(t1, ch_sums[:, 1:2], t1)
            in_v = small.tile([p, 1], f32, tag=tag + "iv", name=tag + "iv")
            nc.vector.tensor_scalar_mul(in_v, t1, 1.0 / n1)
            inv_in = quake_rsqrt(in_v, p, tag + "ii")
            rho = aff_sb[:, 0:1]
            t3 = small.tile([p, 1], f32, tag=tag + "t3", name=tag + "t3")
            nc.vector.tensor_mul(t3, rho, inv_in)
            u1 = small.tile([p, 1], f32, tag=tag + "u1", name=tag + "u1")
            nc.vector.tensor_mul(u1, in_m, t3)
            return t3, u1

        def iln_post(t3, u1, S_col, aff_sb, tag, p=128):
            """Layer stats half + combine -> [p,2] (scale/2, bias/2) for the
            tanh-based activation path."""
            nt, nt1 = N_TOT, N_TOT - 1.0
            ln_m = small.tile([p, 1], f32, tag=tag + "lm", name=tag + "lm")
            nc.vector.tensor_scalar_mul(ln_m, S_col[:, 0:1], 1.0 / nt)
            l1 = small.tile([p, 1], f32, tag=tag + "l1", name=tag + "l1")
            nc.vector.tensor_mul(l1, S_col[:, 0:1], S_col[:, 0:1])
            nc.vector.tensor_scalar_mul(l1, l1, 1.0 / nt)
            nc.vector.tensor_sub(l1, S_col[:, 1:2], l1)
            ln_v = small.tile([p, 1], f32, tag=tag + "lv", name=tag + "lv")
            nc.vector.tensor_scalar_mul(ln_v, l1, 1.0 / nt1)
            inv_ln = quake_rsqrt(ln_v, p, tag + "il")

            rho = aff_sb[:, 0:1]
            t6 = small.tile([p, 1], f32, tag=tag + "t6", name=tag + "t6")
            nc.vector.tensor_mul(t6, rho, inv_ln)
            nc.vector.tensor_sub(t6, inv_ln, t6)
            A = small.tile([p, 1], f32, tag=tag + "A", name=tag + "A")
            nc.vector.tensor_add(A, t3, t6)
            u2 = small.tile([p, 1], f32, tag=tag + "u2", name=tag + "u2")
            nc.vector.tensor_mul(u2, ln_m, t6)
            nc.vector.tensor_add(u2, u1, u2)
            B = small.tile([p, 1], f32, tag=tag + "B", name=tag + "B")
            nc.vector.tensor_scalar_mul(B, u2, -1.0)
            # sb = (gamma*A/2, (gamma*B + beta)/2) -- halved for tanh path
            sb = small.tile([p, 2], f32, tag=tag + "sb", name=tag + "sb")
            nc.vector.tensor_mul(sb[:, 0:1], A, aff_sb[:, 1:2])
            nc.vector.tensor_scalar_mul(sb[:, 0:1], sb[:, 0:1], 0.5)
            nc.vector.tensor_mul(sb[:, 1:2], B, aff_sb[:, 1:2])
            nc.vector.tensor_add(sb[:, 1:2], sb[:, 1:2], aff_sb[:, 2:3])
            nc.vector.tensor_scalar_mul(sb[:, 1:2], sb[:, 1:2], 0.5)
            return sb

        def layer_sums_local(sums2, bounce_d, tag):
            """sums2 [128,2] -> layer [1,2] via a DRAM bounce + DVE free-dim
            reduce (no PSUM, no gpsimd)."""
            nc.sync.dma_start(out=bounce_d[:, :], in_=sums2)
            tr = small.tile([1, 2, 128], f32, tag=tag + "tr", name=tag + "tr")
            nc.sync.dma_start(
                out=tr, in_=bass.AP(tensor=bounce_d, offset=0,
                                    ap=[[0, 1], [1, 2], [2, 128]]))
            lsb = small.tile([1, 2], f32, tag=tag + "lsb", name=tag + "lsb")
            nc.vector.tensor_reduce(out=lsb.rearrange("p (a b) -> p a b", b=1),
                                    in_=tr, axis=mybir.AxisListType.X, op=ALU.add)
            return lsb

        def group_sum_bcast(tr, bounce_d, tag):
            """tr [1, 2, 4] f32 -> reduce over group axis, bounce, broadcast
            to [128, 2]."""
            tsb = small.tile([1, 2], f32, tag=tag + "tsb", name=tag + "tsb")
            nc.vector.tensor_reduce(out=tsb.rearrange("p (a b) -> p a b", b=1),
                                    in_=tr, axis=mybir.AxisListType.X, op=ALU.add)
            nc.sync.dma_start(out=bounce_d[:, :], in_=tsb)
            bc = small.tile([128, 2], f32, tag=tag + "bc", name=tag + "bc")
            nc.sync.dma_start(
                out=bc, in_=bass.AP(tensor=bounce_d, offset=0, ap=[[0, 128], [1, 2]]))
            return bc

        def emit_body():
            attn = ctx.enter_context(tc.tile_pool(name="attn", bufs=1))
            yrep_pool = ctx.enter_context(tc.tile_pool(name="yrep", bufs=1))

            # ---- big input loads issued first: spe alone on the gpsimd
            # queue (nothing may block ahead of the AG triggers later);
            # everything else on sync after ype.
            spe = yrep_pool.tile([128, 130, 130], bf16, tag="spe")
            for rb in range(5):
                r0, r1 = 26 * rb, 26 * rb + 26
                nc.gpsimd.dma_start(out=spe[:, r0:r1, :], in_=spepad_d[:, r0:r1, :])
            yrep = [yrep_pool.tile([128, 66, 66], bf16, tag=f"yrep{kt}",
                                   name=f"yrep{kt}") for kt in range(2)]
            sgate_sb = attn.tile([128, 4096], bf16, tag="sgate")

            # ================= PE warmup (HAM) =================
            warm_src = pers.tile([128, 512], bf16, tag="warmsrc")
            nc.vector.memset(warm_src[:, :], 0.0)
            with tc.tile_pool(name="warmps", bufs=1, space="PSUM") as wps_pool:
                wps = wps_pool.tile([128, 512], f32, tag="warmps")
                for w in range(24):
                    nc.tensor.matmul(wps[:, :], warm_src[:, 0:128], warm_src[:, :],
                                     start=True, stop=True, skip_group_check=True)

            # ================= q conv (tap-outer, 8 banks) =================
            qstats = small.tile([128, 8, 6], f32, tag="qstats")
            qrep = attn.tile([128, 4096], bf16, tag="qrep")
            with tc.tile_pool(name="inp", bufs=1) as inp, \
                 tc.tile_pool(name="qps", bufs=1, space="PSUM") as qps_pool:
                ype = [inp.tile([128, 66, 66], bf16, tag=f"ype{kt}", name=f"ype{kt}")
                       for kt in range(2)]
                for rb in range(3):
                    for kt in range(2):
                        r0, r1 = 22 * rb, 22 * rb + 22
                        eng = nc.sync if kt == 0 else nc.scalar
                        eng.dma_start(out=ype[kt][:, r0:r1, :], in_=ypepad_d[kt][:, r0:r1, :])
                nc.sync.dma_start(out=wg_sb, in_=wg_d[:, :, :].rearrange("t p m -> p t m"))
                nc.sync.dma_start(out=wp_sb, in_=wp_d[:, :, :].rearrange("t p m -> p t m"))
                for kt in range(2):
                    for rb in range(3):
                        r0, r1 = 22 * rb, 22 * rb + 22
                        nc.sync.dma_start(out=yrep[kt][:, r0:r1, :], in_=yreppad_d[kt][:, r0:r1, :])
                nc.sync.dma_start(out=sgate_sb, in_=sgate_d[:, :])

                # 4-way tap-col-tiled: (tap,kt) idx -> col strip idx%4; strips
                # hold partial sums, reduced on DVE afterwards.
                qps = qps_pool.tile([128, 4096], f32, tag="qps")
                strip_last = [16, 17, 14, 15]   # last idx landing in each strip
                for idx in range(18):
                    t, kt = idx // 2, idx % 2
                    ky, kx = t // 3, t % 3
                    st = idx % 4
                    for c in range(8):
                        r0 = 8 * c
                        nc.tensor.matmul(
                            qps[32 * st : 32 * st + 32, 512 * c : 512 * c + 512],
                            wq_sb[:, idx, :],
                            ype[kt][:, r0 + ky : r0 + ky + 8, kx : kx + 64],
                            start=(idx < 4), stop=(idx == strip_last[st]),
                            tile_position=(0, 32 * st),
                            skip_group_check=True,
                        )
                # strip sums -> qsum [32, 4096] bf16 + IN stats
                # (split across DVE and gpsimd to halve the serial chain)
                qsum = attn.tile([32, 4096], bf16, tag="qsum")
                qtmp = small.tile([32, 512], f32, tag="qtmp")
                qtmp2 = small.tile([32, 512], f32, tag="qtmp2")
                for c in range(8):
                    cols = slice(512 * c, 512 * c + 512)
                    t = qtmp if c % 2 == 0 else qtmp2
                    nc.vector.tensor_copy(t, qps[0:32, cols])
                    nc.vector.tensor_add(t, t, qps[32:64, cols])
                    nc.vector.tensor_add(t, t, qps[64:96, cols])
                    nc.vector.tensor_add(qsum[:, cols], t, qps[96:128, cols])
                    nc.vector.bn_stats(out=qstats[0:32, c, :], in_=qsum[:, cols])
                qmv = small.tile([32, 2], f32, tag="qmv")
                nc.vector.bn_aggr(out=qmv, in_=qstats[0:32])
                qinv = quake_rsqrt(qmv[:, 1:2], 32, "qinv")
                qinv2 = small.tile([32, 1], f32, tag="qinv2")
                nc.vector.tensor_scalar_mul(qinv2, qinv, 0.5)
                qbias2 = small.tile([32, 1], f32, tag="qbias2")
                nc.vector.tensor_scalar(out=qbias2, in0=qmv[:, 0:1], scalar1=qinv2,
                                        scalar2=-1.0, op0=ALU.mult, op1=ALU.mult)
                # silu via tanh: q = xq2 * (1 + tanh(xq2)), xq2 = xhat/2
                qth = attn.tile([32, 4096], bf16, tag="qth32")
                xq2 = attn.tile([32, 4096], bf16, tag="xq232")
                for hh in range(2):
                    cols = slice(2048 * hh, 2048 * hh + 2048)
                    nc.scalar.activation(out=qth[:, cols], in_=qsum[:, cols],
                                         func=AF.Tanh, bias=qbias2, scale=qinv2)
                    nc.vector.tensor_scalar(out=xq2[:, cols], in0=qsum[:, cols],
                                            scalar1=qinv2, scalar2=qbias2,
                                            op0=ALU.mult, op1=ALU.add)
                    nc.vector.scalar_tensor_tensor(
                        out=qrep[0:32, cols], in0=qth[:, cols], scalar=1.0,
                        in1=xq2[:, cols], op0=ALU.add, op1=ALU.mult)
                # replicate q to all four 32-row strips for row-tiled QK
                # via SBUF->SBUF DMA (off the DVE queue)
                for rr in range(1, 4):
                    nc.sync.dma_start(out=qrep[32 * rr : 32 * rr + 32, :],
                                      in_=qrep[0:32, :])

            # ================= attention window =================
            greenraw = attn.tile([128, 4096], bf16, tag="greenraw")
            gstats = small.tile([128, 8, 6], f32, tag="gstats")
            vstats = small.tile([32, 8, 6], f32, tag="vstats")
            vraw = attn.tile([32, 4096], bf16, tag="vraw")
            mhapad = attn.tile([128, 66, 66], fp16, tag="mhapad")
            purpleraw = attn.tile([128, 4096], bf16, tag="purpleraw")
            pstats = small.tile([128, 8, 6], f32, tag="pstats")
            # v tap-outer output [128, 32, 48]: (j%128, jb, ch), ch padded to 48
            vtx = attn.tile([128, 32, 48], bf16, tag="vtx")
            vt8 = attn.tile([128, 32, 48], fp8, tag="vt8")
            vt8_pair = vt8.rearrange("p (pr ko) c -> p pr ko c", ko=2)
            gsums2 = None

            with tc.tile_pool(name="vtmp", bufs=2) as vtmp_pool, \
                 tc.tile_pool(name="stage", bufs=2) as stage_pool, \
                 tc.tile_pool(name="aexpp", bufs=12) as aexp_pool, \
                 tc.tile_pool(name="qkps", bufs=1, space="PSUM") as qkps, \
                 tc.tile_pool(name="pvps", bufs=2, space="PSUM") as pvps, \
                 tc.tile_pool(name="fill", bufs=2, space="PSUM") as fillps:
                expbias = small.tile([128, 1], f32, tag="expbias")
                nc.vector.memset(expbias, EXP_BIAS)
                ones_bf = small.tile([1, 32], bf16, tag="onesbf")
                nc.vector.memset(ones_bf, 1.0)

                # ---------- filler machinery ----------
                fillers = []

                def fill_step(n):
                    for _ in range(n):
                        if fillers:
                            fillers.pop(0)()

                # ---- v conv fillers (4-way tap-col-tiled, 3 rounds/chunk;
                # strips summed on DVE into vraw) ----
                V_GROUPS = [(0, 1), (2, 3), (4, 5), (6, 7)]
                V_STRIP_LAST = [8, 5, 6, 7]     # last tap landing in strip tap%4
                vsum_t = small.tile([32, 512], f32, tag="vsumt")

                def make_v_round(grp, gi, tiles_box, rnd):
                    def run():
                        if rnd == 0:
                            tiles_box.clear()
                            tiles_box.extend(
                                fillps.tile([128, 512], f32, tag="fill",
                                            name=f"vps{gi}_{c}") for c in grp)
                        for tap in range(4 * rnd, min(4 * rnd + 4, 9)):
                            ky, kx = tap // 3, tap % 3
                            st = tap % 4
                            for ci, c in enumerate(grp):
                                r0 = 8 * c
                                nc.tensor.matmul(
                                    tiles_box[ci][32 * st : 32 * st + 32, :],
                                    wv_sb[:, tap, :],
                                    spe[:, 2 * r0 + ky : 2 * r0 + ky + 16 : 2, kx : kx + 128 : 2],
                                    start=(tap < 4), stop=(tap == V_STRIP_LAST[st]),
                                    tile_position=(0, 32 * st),
                                    skip_group_check=True,
                                )
                        if rnd == 2:
                            for ci, c in enumerate(grp):
                                cols = slice(512 * c, 512 * c + 512)
                                t = tiles_box[ci]
                                nc.vector.tensor_copy(vsum_t, t[0:32, :])
                                nc.vector.tensor_add(vsum_t, vsum_t, t[32:64, :])
                                nc.vector.tensor_add(vsum_t, vsum_t, t[64:96, :])
                                nc.vector.tensor_add(vraw[:, cols], vsum_t, t[96:128, :])
                                nc.vector.bn_stats(out=vstats[:, c, :], in_=vraw[:, cols])
                    return run

                vbox = []
                for gi, grp in enumerate(V_GROUPS):
                    for rnd in range(3):
                        fillers.append(make_v_round(grp, gi, vbox, rnd))

                vfin_state = {}

                def v_finalize_half(hh):
                    def run():
                        if hh == 0:
                            vmv = small.tile([32, 2], f32, tag="vmv")
                            nc.vector.bn_aggr(out=vmv, in_=vstats)
                            vinv = quake_rsqrt(vmv[:, 1:2], 32, "vinv")
                            vinv2 = small.tile([32, 1], f32, tag="vinv2")
                            nc.vector.tensor_scalar_mul(vinv2, vinv, 0.5)
                            vbias2 = small.tile([32, 1], f32, tag="vbias2")
                            nc.vector.tensor_scalar(out=vbias2, in0=vmv[:, 0:1],
                                                    scalar1=vinv2, scalar2=-1.0,
                                                    op0=ALU.mult, op1=ALU.mult)
                            vfin_state["xv2"] = vtmp_pool.tile([32, 4096], bf16,
                                                               tag="vtmp", name="xv2")
                            vfin_state["vt32"] = vtmp_pool.tile([32, 4096], bf16,
                                                                tag="vtmp", name="vt32")
                            vfin_state["aff"] = (vinv2, vbias2)
                        xv2 = vfin_state["xv2"]; vt32 = vfin_state["vt32"]
                        vinv2, vbias2 = vfin_state["aff"]
                        vt32_v = vt32.rearrange("p (m c) -> p m c", c=32)
                        cols = slice(2048 * hh, 2048 * hh + 2048)
                        nc.vector.tensor_scalar(out=xv2[:, cols], in0=vraw[:, cols],
                                                scalar1=vinv2, scalar2=vbias2,
                                                op0=ALU.mult, op1=ALU.add)
                        nc.vector.transpose(out=vt32[:, cols], in_=xv2[:, cols])
                        for w in range(4):
                            nc.sync.dma_start(
                                out=vtx[32 * w : 32 * w + 32, 16 * hh : 16 * hh + 16, 0:32],
                                in_=vt32_v[:, 64 * hh + w : 64 * hh + 64 : 4, :])
                        # silu via tanh on the transposed half, cast to fp8
                        hv = vtx.rearrange("p a b -> p (a b)")[:, 768 * hh : 768 * hh + 768]
                        h8 = vt8.rearrange("p a b -> p (a b)")[:, 768 * hh : 768 * hh + 768]
                        tv = vtmp_pool.tile([128, 768], bf16, tag="vth", name=f"tv{hh}")
                        nc.scalar.activation(out=tv, in_=hv, func=AF.Tanh)
                        with nc.allow_low_precision(reason="fp8 v for DoubleRow PV"):
                            nc.vector.scalar_tensor_tensor(
                                out=h8, in0=tv, scalar=1.0, in1=hv,
                                op0=ALU.add, op1=ALU.mult)
                            nc.vector.memset(
                                vt8[:, 16 * hh : 16 * hh + 16, 32:33], 1.0)
                    return run

                fillers.append(v_finalize_half(0))
                fillers.append(v_finalize_half(1))

                # ---- green conv fillers (phase-col-tiled: 8 rounds of 4
                # concurrent phase MMs per chunk; phase p owns partitions
                # 32p..32p+32 of the psum = its 32 output channels) ----
                G_GROUPS = [(0, 1), (2, 3), (4, 5), (6, 7)]

                def make_g_round(grp, gi, tiles_box, rnd):
                    # rnd = (dy*2+dx)*2 + kt in 0..7
                    def run():
                        if rnd == 0:
                            tiles_box.clear()
                            tiles_box.extend(
                                fillps.tile([128, 512], f32, tag="fill",
                                            name=f"gps{gi}_{c}") for c in grp)
                        dy, dx = rnd // 4, (rnd // 2) % 2
                        kt = rnd % 2
                        for p in range(4):
                            pr, pc = p // 2, p % 2
                            ey, ex = pr + dy, pc + dx
                            for ci, c in enumerate(grp):
                                r0 = 8 * c
                                nc.tensor.matmul(
                                    tiles_box[ci][32 * p : 32 * p + 32, :],
                                    wg_sb[:, 4 * rnd + p, :],
                                    yrep[kt][:, r0 + ey : r0 + ey + 8, ex : ex + 64],
                                    start=(rnd == 0), stop=(rnd == 7),
                                    tile_position=(0, 32 * p),
                                    skip_group_check=True,
                                )
                        if rnd == 7:
                            for ci, c in enumerate(grp):
                                cols = slice(512 * c, 512 * c + 512)
                                nc.vector.tensor_copy(greenraw[:, cols], tiles_box[ci][:, :])
                                nc.vector.bn_stats(out=gstats[:, c, :], in_=greenraw[:, cols])
                    return run

                gbox = []
                for gi, grp in enumerate(G_GROUPS):
                    for rnd in range(8):
                        fillers.append(make_g_round(grp, gi, gbox, rnd))

                # ---- purple conv fillers (phase-col-tiled, 4 rounds) ----
                def make_p_round(grp, gi, tiles_box, rnd):
                    # rnd = dy*2+dx in 0..3
                    def run():
                        if rnd == 0:
                            tiles_box.clear()
                            tiles_box.extend(
                                fillps.tile([128, 512], f32, tag="fill",
                                            name=f"pps{gi}_{c}") for c in grp)
                        dy, dx = rnd // 2, rnd % 2
                        for p in range(4):
                            pr, pc = p // 2, p % 2
                            ey, ex = pr + dy, pc + dx
                            for ci, c in enumerate(grp):
                                r0 = 8 * c
                                nc.tensor.matmul(
                                    tiles_box[ci][32 * p : 32 * p + 32, :],
                                    wp_sb[:, 4 * rnd + p, :],
                                    mhapad[:, r0 + ey : r0 + ey + 8, ex : ex + 64],
                                    start=(rnd == 0), stop=(rnd == 3),
                                    tile_position=(0, 32 * p),
                                    skip_group_check=True,
                                )
                        if rnd == 3:
                            for ci, c in enumerate(grp):
                                cols = slice(512 * c, 512 * c + 512)
                                nc.vector.tensor_copy(purpleraw[:, cols], tiles_box[ci][:, :])
                                nc.vector.bn_stats(out=pstats[:, c, :], in_=purpleraw[:, cols])
                    return run

                pbox = []

                def queue_purple(grp, gi):
                    for rnd in range(4):
                        fillers.append(make_p_round(grp, gi, pbox, rnd))

                def emit_divide(k):
                    """Pull AG chunk k's gathered mha rows into mhapad + edge pad.
                    Everything on the gpsimd queue: these ops gate on the
                    AllGather and must not block the DVE/PE/Sync queues."""
                    c0, c1 = AG_BOUNDS[k]
                    w = c1 - c0
                    r0, r1 = c0 // 64, c1 // 64       # image rows
                    nc.gpsimd.dma_start(
                        out=mhapad[:, r0 + 1 : r1 + 1, 1:65],
                        in_=bass.AP(tensor=ccout[k], offset=0,
                                    ap=[[33 * w, 4], [w, 32], [1, w]]))
                    nc.gpsimd.tensor_copy(
                        mhapad[:, r0 + 1 : r1 + 1, 0:1],
                        mhapad[:, r0 + 1 : r1 + 1, 1:2])
                    nc.gpsimd.tensor_copy(
                        mhapad[:, r0 + 1 : r1 + 1, 65:66],
                        mhapad[:, r0 + 1 : r1 + 1, 64:65])
                    if k == 0:
                        nc.gpsimd.tensor_copy(mhapad[:, 0:1, :], mhapad[:, 1:2, :])
                    if k == len(AG_BOUNDS) - 1:
                        nc.gpsimd.tensor_copy(mhapad[:, 65:66, :], mhapad[:, 64:65, :])

                # ---------- the attention I/g loop ----------
                ag_ends = {c1 // 512 - 1: k for k, (c0, c1) in enumerate(AG_BOUNDS)}
                pv_flush = []   # deferred PV work for I=0

                def emit_pv(pvt, g, aexp_t, first, last):
                    """Two fp8 DoubleRow MMs: pairs (2g, 2g+1) into pvt."""
                    for pi in range(2):
                        pair = 2 * g + pi
                        nc.tensor.matmul(
                            pvt[0:33, :],
                            vt8_pair[:, pair, :, 0:33],
                            aexp_t[:, pi, :, :],
                            start=(first and pi == 0), stop=(last and pi == 1),
                            perf_mode=DR,
                            skip_group_check=True,
                        )

                def emit_div_chain(I, pvt):
                    """Reciprocal + broadcast + stage + ccin DMA + AG send for
                    I-block I.  Deferred into the next I's units so the DVE/PE
                    queues never stall on it at an I boundary."""
                    rec = schraudolph_recip(
                        pvt[32:33, :], 512, stage_pool, f"rec{I}",
                        out_bf=stage_pool.tile([1, 512], bf16, tag="recbf",
                                               name=f"recbf{I}"))
                    nc.tensor.matmul(pvt[64:96, :], ones_bf, rec,
                                     start=True, stop=True, skip_group_check=True)
                    rbs = stage_pool.tile([32, 512], f32, tag="rbs", name=f"rbs{I}")
                    nc.vector.tensor_copy(rbs, pvt[64:96, :])
                    stg = stage_pool.tile([32, 512], fp16, tag="stage", name=f"stg{I}")
                    with nc.allow_low_precision(reason="fp16 attention output"):
                        nc.vector.tensor_mul(stg, pvt[0:32, :], rbs)
                    sk = next(k for k, (c0, c1) in enumerate(AG_BOUNDS)
                              if c0 <= 512 * I < c1)
                    lc = 512 * I - AG_BOUNDS[sk][0]
                    nc.sync.dma_start(out=ccin[sk][0:32, lc : lc + 512], in_=stg)

                    if ag_ends.get(I) == AG_GREEN:
                        # green complete by now: in-band layer sums, same chunk
                        gmv = small.tile([128, 2], f32, tag="gmv")
                        nc.vector.bn_aggr(out=gmv, in_=gstats)
                        nonlocal gsums2
                        gsums2 = part_sums(gmv, "gsums2")
                        glsb = layer_sums_local(gsums2, psum_b, "gl")
                        nc.sync.dma_start(out=ccin[AG_GREEN][32:33, 0:4],
                                          in_=glsb.bitcast(fp16))
                    if I in ag_ends:
                        k = ag_ends[I]
                        if no_cc:
                            for g4 in range(4):
                                nc.sync.dma_start(
                                    out=ccout[k][33 * g4 : 33 * g4 + 33, :],
                                    in_=ccin[k][:, :])
                        else:
                            nc.gpsimd.collective_compute(
                                "AllGather", ALU.bypass,
                                replica_groups=_REPLICA_GROUPS,
                                ins=[ccin[k][:, :]],
                                outs=[ccout[k][:, :]],
                            )

                pending_div = None
                fill_step(12)   # v conv runs while the q stats/silu chain computes
                for I in range(8):
                    pvt = pvps.tile([128, 512], f32, tag="pvt", name=f"pvt{I}")
                    for g in range(8):
                        if g == 2 and pending_div is not None:
                            emit_div_chain(*pending_div)
                            pending_div = None
                        # QK half-tiles pipeline against the other half's exp:
                        # QK r0/r1 wait only on exp_lo(g-1), r2/r3 on exp_hi(g-1).
                        qk4 = qkps.tile([128, 2048], f32, tag="qk4", name=f"qk4_{I}_{g}")
                        for r in range(4):
                            jb = 4 * g + r
                            nc.tensor.matmul(
                                qk4[:, 512 * r : 512 * r + 512],
                                qrep[32 * r : 32 * r + 32, 128 * jb : 128 * jb + 128],
                                qrep[32 * r : 32 * r + 32, 512 * I : 512 * I + 512],
                                start=True, stop=True,
                                tile_position=(32 * r, 0),
                                skip_group_check=True,
                            )
                        aexp = aexp_pool.tile([128, 2048], fp8, tag="aexp",
                                              name=f"ae{I}_{g}")
                        with nc.allow_low_precision(reason="fp8 attention weights"):
                            nc.scalar.activation(out=aexp[:, 0:1024], in_=qk4[:, 0:1024],
                                                 func=AF.Exp, scale=SCALE, bias=expbias)
                            nc.scalar.activation(out=aexp[:, 1024:2048], in_=qk4[:, 1024:2048],
                                                 func=AF.Exp, scale=SCALE, bias=expbias)
                        aexp_t = aexp.rearrange("p (pr ko i) -> p pr ko i", ko=2, i=512)
                        pv_flush.append((pvt, g, aexp_t, g == 0, g == 7))
                        if I > 0 or g == 7:
                            # steady state: emit PV one unit late so it never
                            # stalls the PE queue head on a fresh exp
                            while len(pv_flush) > (1 if g < 7 else 0):
                                args = pv_flush.pop(0)
                                emit_pv(args[0], args[1], args[2], args[3], args[4])
                        fill_step(1 if I > 0 else 2)

                    pending_div = (I, pvt)
                    # purple fillers as AG chunks land
                    if I == 5:
                        emit_divide(0)
                        emit_divide(1)
                    if I == 6:
                        queue_purple((0, 1), 0)

                emit_div_chain(*pending_div)
                pending_div = None

                # ---- green finalize (gates only on AG2 + gsums2; overlaps
                # the tail AGs) ----
                gx = small.tile([1, 16], fp16, tag="gx")
                w1 = AG_BOUNDS[AG_GREEN][1] - AG_BOUNDS[AG_GREEN][0]
                nc.gpsimd.dma_start(
                    out=gx,
                    in_=bass.AP(tensor=ccout[AG_GREEN], offset=32 * w1,
                                ap=[[2, 2], [33 * w1, 4], [1, 2]]))
                glsum = group_sum_bcast(
                    gx.bitcast(f32).rearrange("p (k g) -> p k g", g=4), gls_b, "gl")
                gfill = fillps.tile([128, 512], f32, tag="fill", name="gch_ps")
                nc.tensor.matmul(gfill[:, 0:2], redmats_sb[:, 0:128], gsums2,
                                 start=True, stop=True, skip_group_check=True)
                gch = small.tile([128, 2], f32, tag="gach", name="gach")
                nc.vector.tensor_copy(gch, gfill[:, 0:2])
                gt3, gu1 = iln_local(gch, affg_sb, "ga")
                gsb = iln_post(gt3, gu1, glsum, affg_sb, "ga")
                # upy = silu(A*x+B) = u2*(1+tanh(u2)), u2 = x*A/2 + B/2
                # (reuse the q-phase slabs: those tiles are dead by now)
                gth = attn.tile([128, 4096], bf16, tag="qth32", name="gth")
                gu2 = attn.tile([128, 4096], bf16, tag="xq232", name="gu2")
                upy_sb = attn.tile([128, 4096], bf16, tag="qsum", name="upy")
                for hh in range(2):
                    cols = slice(2048 * hh, 2048 * hh + 2048)
                    nc.scalar.activation(out=gth[:, cols], in_=greenraw[:, cols],
                                         func=AF.Tanh, bias=gsb[:, 1:2], scale=gsb[:, 0:1])
                    nc.vector.tensor_scalar(out=gu2[:, cols], in0=greenraw[:, cols],
                                            scalar1=gsb[:, 0:1], scalar2=gsb[:, 1:2],
                                            op0=ALU.mult, op1=ALU.add)
                    nc.vector.scalar_tensor_tensor(
                        out=upy_sb[:, cols], in0=gth[:, cols], scalar=1.0,
                        in1=gu2[:, cols], op0=ALU.add, op1=ALU.mult)
                    nc.sync.dma_start(out=upyout_d[:, cols], in_=upy_sb[:, cols])

                emit_divide(2)
                queue_purple((2, 3), 1)
                fill_step(len(fillers))
                emit_divide(3)
                queue_purple((4, 5), 2)
                emit_divide(4)
                queue_purple((6, 7), 3)
                fill_step(len(fillers))

                # ---- purple stats + AG + ILN ----
                pmv = small.tile([128, 2], f32, tag="pmv")
                nc.vector.bn_aggr(out=pmv, in_=pstats)
                psums2 = part_sums(pmv, "psums2")
                nc.sync.dma_start(out=psum_b2[:, :], in_=psums2)
                if no_cc:
                    for g4 in range(4):
                        nc.sync.dma_start(out=cp_out4[128 * g4 : 128 * g4 + 128, :],
                                          in_=psum_b2[:, :])
                else:
                    nc.gpsimd.collective_compute(
                        "AllGather", ALU.bypass,
                        replica_groups=_REPLICA_GROUPS,
                        ins=[bass.AP(tensor=psum_b2, offset=0, ap=[[2, 128], [1, 2]])],
                        outs=[bass.AP(tensor=cp_out4, offset=0, ap=[[2, 512], [1, 2]])],
                    )
                pfill = fillps.tile([128, 512], f32, tag="fill", name="pch_ps")
                nc.tensor.matmul(pfill[:, 0:2], redmats_sb[:, 0:128], psums2,
                                 start=True, stop=True, skip_group_check=True)
                pch = small.tile([128, 2], f32, tag="pach", name="pach")
                nc.vector.tensor_copy(pch, pfill[:, 0:2])
                pt3, pu1 = iln_local(pch, affp_sb, "pa")
                ptr = small.tile([128, 2, 4], f32, tag="ptr")
                nc.sync.dma_start(
                    out=ptr, in_=bass.AP(tensor=cp_out4, offset=0,
                                         ap=[[2, 128], [1, 2], [256, 4]]))
                pfill2 = fillps.tile([128, 512], f32, tag="fill", name="pls_ps")
                nc.tensor.matmul(pfill2[:, 0:8],
                                 redmats_sb[:, 128:256],
                                 ptr.rearrange("p a b -> p (a b)"),
                                 start=True, stop=True, skip_group_check=True)
                pls4 = small.tile([128, 2, 4], f32, tag="pls4", name="pls4")
                nc.vector.tensor_copy(pls4.rearrange("p a b -> p (a b)"), pfill2[:, 0:8])
                plsum = small.tile([128, 2], f32, tag="plsum")
                nc.vector.tensor_reduce(out=plsum.rearrange("p (a b) -> p a b", b=1),
                                        in_=pls4, axis=mybir.AxisListType.X, op=ALU.add)
                psb = iln_post(pt3, pu1, plsum, affp_sb, "pa")

                # z = sigmoid(A*x+B)*s = (1+tanh(u2))*(s/2), u2 = x*A/2+B/2
                pth = attn.tile([128, 4096], bf16, tag="qth32", name="pth")
                zpre = attn.tile([128, 4096], bf16, tag="xq232", name="zpre")
                for hh in range(4):
                    cols = slice(1024 * hh, 1024 * hh + 1024)
                    nc.scalar.activation(out=pth[:, cols], in_=purpleraw[:, cols],
                                         func=AF.Tanh, bias=psb[:, 1:2], scale=psb[:, 0:1])
                    nc.vector.scalar_tensor_tensor(
                        out=zpre[:, cols], in0=pth[:, cols], scalar=1.0,
                        in1=sgate_sb[:, cols], op0=ALU.add, op1=ALU.mult)
                    nc.sync.dma_start(out=zout_d[:, cols], in_=zpre[:, cols])

        emit_body()

    nc.compile()
    return nc


_NC_CACHE = None
RUN_KWARGS = {}      # test harness may set e.g. {"trace": True}
LAST_RESULTS = None  # BassKernelResults of the most recent run


def kernel(**inputs) -> np.ndarray:
    global _NC_CACHE, LAST_RESULTS
    from concourse.bass_utils import run_bass_kernel_spmd

    if _NC_CACHE is None:
        _NC_CACHE = build_bass()
    nc = _NC_CACHE

    in_maps = []
    for core in _CORES:
        ci = prepare_core_inputs(inputs, core)
        in_maps.append(ci)

    res = run_bass_kernel_spmd(nc, in_maps, _CORES, **RUN_KWARGS)
    LAST_RESULTS = res
    zs = [res.results[c]["zout"] for c in _CORES]
    upys = [res.results[c]["upyout"] for c in _CORES]
    return assemble_output(zs, upys)


if __name__ == "__main__":
    nc = build_bass()
    print("bass build OK")


# revision 46
# speedup vs baseline: 1.1621x; 1.1291x over previous
"""Trainium2 Bass kernel for nn_MultiHeadCrossAttention_47519518163418.

Sharding: 8 cores = (batch b in {0,1}) x (head h in {0..3}); core c: b=c//4, h=c%4.

v2 design (ACT-exp-bound window, PE fillers):
 - q conv: tap-outer over all 8 PSUM banks, weights replicated 4x on the
   output dim so qrep [128,4096] bf16 comes out partition-replicated.
 - QK: bf16, 4-way row-tiled (tile_position (32r,0)) -> 4 score blocks
   [128,512] land in one [128,2048] PSUM tile per (I,g).
 - exp: one N=2048 ACT (Exp, scale=1/sqrt(32), bias=-2.5) writing fp8-e4m3
   directly in DoubleRow-interleaved layout.  Softmax is invariant to the
   exp bias (cancels in the division).
 - PV: fp8 DoubleRow matmuls (256-row contraction, 2 per (I,g)), v in fp8
   with a ones column for the row sums.
 - All silu/sigmoid via tanh (same ACT table set as exp -> zero table
   swaps); rsqrt/recip via int-bit-trick + Newton on DVE (no ACT sqrt).
 - green/v/purple convs run as paced PE "fillers" inside the exp window;
   the AllGather flow (5 mha chunks + in-band green layer sums + purple
   stat gather) follows the previous kernel.
"""

import sys

if "/opt/trn_rl_repo" not in sys.path:
    sys.path.insert(0, "/opt/trn_rl_repo")

import numpy as np
import ml_dtypes

BF16 = ml_dtypes.bfloat16

NUM_HEADS = 4
EPS = 1e-5
D_HEAD = 32
SCALE = float(D_HEAD) ** -0.5
EXP_BIAS = -2.5
N_PX = 16384.0          # pixels per channel of the upsampled image
N_TOT = 128 * 16384.0   # elements per batch for layer stats

_CORES = list(range(8))
_REPLICA_GROUPS = [[0, 1, 2, 3], [4, 5, 6, 7]]

# int-bit-trick constants
RSQRT_MAGIC = float(0x5F3759DF)
RECIP_MAGIC = float(0x7EF127EA)


# ----------------------------------------------------------------------------
# Host-side helpers
# ----------------------------------------------------------------------------

def pos_encoding_pe(c, L, dtype=np.float32):
    half = c // 2
    pos = np.arange(L, dtype=dtype)
    depths = np.arange(half, dtype=dtype) / half
    rates = 1.0 / (10000.0 ** depths)
    ang = pos[:, None] * rates[None, :]
    pe = np.concatenate([np.sin(ang), np.cos(ang)], axis=-1)  # [L, c]
    return pe.T.astype(dtype)  # [c, L]


def reflect_pad(x):
    return np.pad(x, ((0, 0), (1, 1), (1, 1)), mode="reflect")


def edge_pad(x):
    return np.pad(x, ((0, 0), (1, 1), (1, 1)), mode="edge")


_KSET = {(0, 0): [0], (0, 1): [1, 2], (1, 0): [0, 1], (1, 1): [2]}


def collapse_w2(w):
    """w [co, ci, 3, 3] -> W2 [4 (p=2*pr+pc), 2 (dy), 2 (dx), ci, co]."""
    co, ci = w.shape[0], w.shape[1]
    W2 = np.zeros((4, 2, 2, ci, co), dtype=w.dtype)
    for pr in range(2):
        for pc in range(2):
            p = 2 * pr + pc
            for dy in range(2):
                for dx in range(2):
                    acc = np.zeros((co, ci), dtype=np.float64)
                    for ky in _KSET[(pr, dy)]:
                        for kx in _KSET[(pc, dx)]:
                            acc = acc + w[:, :, ky, kx].astype(np.float64)
                    W2[p, dy, dx] = acc.T.astype(w.dtype)
    return W2


def arrange_to_strips(x2d):
    """x [32, 128, 128] -> arranged [128, 4096] phase-major: partition
    32*(2*pr+pc)+c, free r*64+cc for upsampled pixel (2r+pr, 2cc+pc)."""
    t = x2d.reshape(32, 64, 2, 64, 2)          # c, r, pr, cc, pc
    t = t.transpose(2, 4, 0, 1, 3)              # pr, pc, c, r, cc
    return np.ascontiguousarray(t.reshape(128, 4096))


def unarrange_from_strips(arr):
    t = arr.reshape(2, 2, 32, 64, 64)           # pr, pc, c, r, cc
    t = t.transpose(2, 3, 0, 4, 1)              # c, r, pr, cc, pc
    return np.ascontiguousarray(t.reshape(32, 128, 128))


_PE_Y = None
_PE_S = None


_BATCH_CACHE = {}


def _batch_shared(inputs, b):
    """Padded/PE-added tensors shared by the 4 cores of a batch."""
    key = (id(inputs), b)
    if key in _BATCH_CACHE:
        return _BATCH_CACHE[key]
    y = np.asarray(inputs["y"], dtype=np.float32)[b]
    s = np.asarray(inputs["s"], dtype=np.float32)[b]
    ypepad = np.ascontiguousarray(
        reflect_pad((y + _PE_Y).astype(np.float32)).reshape(2, 128, 66, 66)
    ).astype(BF16)
    yreppad = np.ascontiguousarray(edge_pad(y).reshape(2, 128, 66, 66)).astype(BF16)
    spepad = np.ascontiguousarray(reflect_pad((s + _PE_S).astype(np.float32))).astype(BF16)
    _BATCH_CACHE.clear()
    _BATCH_CACHE[key] = (ypepad, yreppad, spepad)
    return _BATCH_CACHE[key]


def prepare_core_inputs(inputs, core):
    global _PE_Y, _PE_S
    if _PE_Y is None:
        _PE_Y = pos_encoding_pe(256, 64 * 64).reshape(256, 64, 64)
        _PE_S = pos_encoding_pe(128, 128 * 128).reshape(128, 128, 128)
    b, h = core // 4, core % 4
    ch = slice(32 * h, 32 * h + 32)
    s = np.asarray(inputs["s"], dtype=np.float32)[b]

    ypepad, yreppad, spepad = _batch_shared(inputs, b)
    sgate = (arrange_to_strips(np.ascontiguousarray(s[ch])) * 0.5).astype(BF16)

    w_blue_y = np.asarray(inputs["w_blue_y"], dtype=np.float32)[ch]
    w_blue_s = np.asarray(inputs["w_blue_s"], dtype=np.float32)[ch]
    w_green = np.asarray(inputs["w_green"], dtype=np.float32)[ch]
    w_purple = np.asarray(inputs["w_purple"], dtype=np.float32)[ch]

    # q conv weights [18 (tap,kt), 128ci, 32co] (col-tiled by tap on device)
    wq = np.zeros((18, 128, 32), dtype=np.float32)
    for t in range(9):
        ky, kx = t // 3, t % 3
        for kt in range(2):
            wq[t * 2 + kt] = w_blue_y[:, 128 * kt : 128 * kt + 128, ky, kx].T
    wv = np.zeros((9, 128, 32), dtype=np.float32)
    for t in range(9):
        ky, kx = t // 3, t % 3
        wv[t] = w_blue_s[:, :, ky, kx].T

    # green/purple: per-phase collapsed weights (no zero padding) for 4-way
    # column-tiled conv: phase p writes psum partitions 32p..32p+32.
    W2g = collapse_w2(w_green)                   # [4, 2, 2, 256, 32]
    wg = np.zeros((32, 128, 32), dtype=np.float32)
    for p in range(4):
        for dy in range(2):
            for dx in range(2):
                for kt in range(2):
                    idx = ((dy * 2 + dx) * 2 + kt) * 4 + p
                    wg[idx] = W2g[p, dy, dx][128 * kt : 128 * kt + 128, :]
    W2p = collapse_w2(w_purple)                  # [4, 2, 2, 128, 32]
    wp = np.zeros((16, 128, 32), dtype=np.float32)
    for p in range(4):
        for dy in range(2):
            for dx in range(2):
                idx = (dy * 2 + dx) * 4 + p
                wp[idx] = W2p[p, dy, dx]

    affg = np.tile(np.stack(
        [np.asarray(inputs["rho_g"], dtype=np.float32).reshape(128)[ch],
         np.asarray(inputs["gamma_g"], dtype=np.float32).reshape(128)[ch],
         np.asarray(inputs["beta_g"], dtype=np.float32).reshape(128)[ch]],
        axis=1), (4, 1))
    affp = np.tile(np.stack(
        [np.asarray(inputs["rho_p"], dtype=np.float32).reshape(128)[ch],
         np.asarray(inputs["gamma_p"], dtype=np.float32).reshape(128)[ch],
         np.asarray(inputs["beta_p"], dtype=np.float32).reshape(128)[ch]],
        axis=1), (4, 1))

    pmat = np.zeros((128, 128), dtype=np.float32)
    for k in range(4):
        for a in range(4):
            pmat[32 * k : 32 * k + 32, 32 * a : 32 * a + 32] = np.eye(32, dtype=np.float32)
    redmats = np.concatenate([pmat, np.ones((128, 128), np.float32)], axis=1)

    return {
        "ypepad": ypepad,
        "redmats": redmats,
        "yreppad": yreppad,
        "spepad": spepad,
        "sgate": sgate,
        "wq": wq.astype(BF16),
        "wv": wv.astype(BF16),
        "wg": wg.astype(BF16),
        "wp": wp.astype(np.float16),
        "affg": np.ascontiguousarray(affg),
        "affp": np.ascontiguousarray(affp),
    }


def assemble_output(per_core_z, per_core_upy):
    out = np.zeros((2, 256, 128, 128), dtype=np.float32)
    for core in range(8):
        b, h = core // 4, core % 4
        z = np.asarray(per_core_z[core]).astype(np.float32)
        u = np.asarray(per_core_upy[core]).astype(np.float32)
        out[b, 32 * h : 32 * h + 32] = unarrange_from_strips(z)
        out[b, 128 + 32 * h : 128 + 32 * h + 32] = unarrange_from_strips(u)
    return out


# ----------------------------------------------------------------------------
# Bass kernel
# ----------------------------------------------------------------------------

def build_bass(no_cc=False):
    import concourse.bass as bass
    import concourse.tile as tile
    from concourse import bacc, mybir

    f32 = mybir.dt.float32
    i32 = mybir.dt.int32
    bf16 = mybir.dt.bfloat16
    fp16 = mybir.dt.float16
    fp8 = mybir.dt.float8e4
    AF = mybir.ActivationFunctionType
    ALU = mybir.AluOpType
    DR = mybir.MatmulPerfMode.DoubleRow

    nc = bacc.Bacc(num_devices=8)

    # ---- I/O ----
    ypepad_d = nc.declare_dram_parameter("ypepad", [2, 128, 66, 66], bf16, isOutput=False)
    yreppad_d = nc.declare_dram_parameter("yreppad", [2, 128, 66, 66], bf16, isOutput=False)
    spepad_d = nc.declare_dram_parameter("spepad", [128, 130, 130], bf16, isOutput=False)
    sgate_d = nc.declare_dram_parameter("sgate", [128, 4096], bf16, isOutput=False)
    wq_d = nc.declare_dram_parameter("wq", [18, 128, 32], bf16, isOutput=False)
    wv_d = nc.declare_dram_parameter("wv", [9, 128, 32], bf16, isOutput=False)
    wg_d = nc.declare_dram_parameter("wg", [32, 128, 32], bf16, isOutput=False)
    wp_d = nc.declare_dram_parameter("wp", [16, 128, 32], fp16, isOutput=False)
    redmats_d = nc.declare_dram_parameter("redmats", [128, 256], f32, isOutput=False)
    affg_d = nc.declare_dram_parameter("affg", [128, 3], f32, isOutput=False)
    affp_d = nc.declare_dram_parameter("affp", [128, 3], f32, isOutput=False)
    zout_d = nc.declare_dram_parameter("zout", [128, 4096], bf16, isOutput=True)
    upyout_d = nc.declare_dram_parameter("upyout", [128, 4096], bf16, isOutput=True)

    # ---- internal DRAM (collective bounce buffers etc.) ----
    AG_BOUNDS = [(0, 1024), (1024, 2048), (2048, 3072),
                 (3072, 3584), (3584, 4096)]
    ccin = [nc.dram_tensor(f"ccin{k}", [33, c1 - c0], fp16)
            for k, (c0, c1) in enumerate(AG_BOUNDS)]
    ccout = [nc.dram_tensor(f"ccout{k}", [132, c1 - c0], fp16)
             for k, (c0, c1) in enumerate(AG_BOUNDS)]
    AG_GREEN = 2  # chunk whose payload carries green layer sums in-band
    psum_b = nc.dram_tensor("psum_b", [128, 2], f32)    # partition-sum bounce
    psum_b2 = nc.dram_tensor("psum_b2", [128, 2], f32)
    cp_out4 = nc.dram_tensor("cp_out4", [512, 2], f32)
    gls_b = nc.dram_tensor("gls_b", [1, 2], f32)        # layer-sum bounce (green)

    import contextlib

    with tile.TileContext(nc) as tc, contextlib.ExitStack() as ctx:
        pers = ctx.enter_context(tc.tile_pool(name="pers", bufs=1))
        small = ctx.enter_context(tc.tile_pool(name="small", bufs=1))

        # ---------------- weights + constants ----------------
        wq_sb = pers.tile([128, 18, 32], bf16, tag="wq")
        nc.sync.dma_start(out=wq_sb, in_=wq_d[:, :, :].rearrange("t p m -> p t m"))
        wv_sb = pers.tile([128, 9, 32], bf16, tag="wv")
        nc.sync.dma_start(out=wv_sb, in_=wv_d[:, :, :].rearrange("t p m -> p t m"))
        wg_sb = pers.tile([128, 32, 32], bf16, tag="wg")
        wp_sb = pers.tile([128, 16, 32], fp16, tag="wp")
        redmats_sb = pers.tile([128, 256], f32, tag="redmats")
        nc.sync.dma_start(out=redmats_sb, in_=redmats_d[:, :])
        affg_sb = small.tile([128, 3], f32, tag="affg")
        nc.sync.dma_start(out=affg_sb, in_=affg_d[:, :])
        affp_sb = small.tile([128, 3], f32, tag="affp")
        nc.sync.dma_start(out=affp_sb, in_=affp_d[:, :])

        def quake_rsqrt(x, p, tag, eps=EPS, steps=3):
            """[p,1] f32 -> rsqrt(x+eps) via 0x5F3759DF bit trick + Newton.
            DVE only; no ACT table involvement."""
            xe = small.tile([p, 1], f32, tag=tag + "xe", name=tag + "xe")
            nc.vector.tensor_scalar_add(xe, x, eps)
            xh = small.tile([p, 1], f32, tag=tag + "xh", name=tag + "xh")
            nc.vector.tensor_scalar_mul(xh, xe, 0.5)
            yi = small.tile([p, 1], i32, tag=tag + "yi", name=tag + "yi")
            nc.vector.tensor_scalar(out=yi, in0=xe.bitcast(i32), scalar1=-0.5,
                                    scalar2=RSQRT_MAGIC, op0=ALU.mult, op1=ALU.add)
            y = yi.bitcast(f32)
            t = small.tile([p, 1], f32, tag=tag + "t", name=tag + "t")
            for _ in range(steps):
                nc.vector.tensor_mul(t, y, y)
                nc.vector.tensor_mul(t, t, xh)
                nc.vector.tensor_scalar(out=t, in0=t, scalar1=-1.0, scalar2=1.5,
                                        op0=ALU.mult, op1=ALU.add)
                nc.vector.tensor_mul(y, y, t)
            return y

        def schraudolph_recip(x, cols, pool, name, steps=2, out_bf=None):
            """[1,cols] f32 (psum ok) -> 1/x via 0x7EF127EA bit trick + Newton.
            If out_bf is given, the last Newton product lands there (bf16)."""
            yi = pool.tile([1, cols], i32, tag="recyi", name=name + "yi")
            nc.vector.tensor_scalar(out=yi, in0=x.bitcast(i32), scalar1=-1.0,
                                    scalar2=RECIP_MAGIC, op0=ALU.mult, op1=ALU.add)
            y = yi.bitcast(f32)
            t = pool.tile([1, cols], f32, tag="rect", name=name + "t")
            for s in range(steps):
                nc.vector.tensor_mul(t, x, y)
                nc.vector.tensor_scalar(out=t, in0=t, scalar1=-1.0, scalar2=2.0,
                                        op0=ALU.mult, op1=ALU.add)
                if s == steps - 1 and out_bf is not None:
                    nc.vector.tensor_mul(out_bf, y, t)
                    return out_bf
                nc.vector.tensor_mul(y, y, t)
            return y

        def part_sums(mv, tag):
            """[128,2] per-partition (mean, biased var over 4096) ->
            [128,2] (sum, sum of squares over the 4096 pixels)."""
            s2 = small.tile([128, 2], f32, tag=tag, name=tag)
            nc.vector.tensor_scalar_mul(s2[:, 0:1], mv[:, 0:1], 4096.0)
            t = small.tile([128, 1], f32, tag=tag + "t", name=tag + "t")
            nc.vector.tensor_mul(t, mv[:, 0:1], mv[:, 0:1])
            nc.vector.tensor_add(t, t, mv[:, 1:2])
            nc.vector.tensor_scalar_mul(s2[:, 1:2], t, 4096.0)
            return s2

        def iln_local(ch_sums, aff_sb, tag, p=128):
            """Per-channel (sum, sumsq) [p,2] -> (t3 = rho*inv_in, u1 = in_m*t3)."""
            n, n1 = N_PX, N_PX - 1.0
            in_m = small.tile([p, 1], f32, tag=tag + "im", name=tag + "im")
            nc.vector.tensor_scalar_mul(in_m, ch_sums[:, 0:1], 1.0 / n)
            t1 = small.tile([p, 1], f32, tag=tag + "t1", name=tag + "t1")
            nc.vector.tensor_mul(t1, ch_sums[:, 0:1], ch_sums[:, 0:1])
            nc.vector.tensor_scalar_mul(t1, t1, 1.0 / n)
            nc.vector.tensor_sub(t1, ch_sums[:, 1:2], t1)
            in_v = small.tile([p, 1], f32, tag=tag + "iv", name=tag + "iv")
            nc.vector.tensor_scalar_mul(in_v, t1, 1.0 / n1)
            inv_in = quake_rsqrt(in_v, p, tag + "ii")
            rho = aff_sb[:, 0:1]
            t3 = small.tile([p, 1], f32, tag=tag + "t3", name=tag + "t3")
            nc.vector.tensor_mul(t3, rho, inv_in)
            u1 = small.tile([p, 1], f32, tag=tag + "u1", name=tag + "u1")
            nc.vector.tensor_mul(u1, in_m, t3)
            return t3, u1

        def iln_post(t3, u1, S_col, aff_sb, tag, p=128):
            """Layer stats half + combine -> [p,2] (scale/2, bias/2) for the
            tanh-based activation path."""
            nt, nt1 = N_TOT, N_TOT - 1.0
            ln_m = small.tile([p, 1], f32, tag=tag + "lm", name=tag + "lm")
            nc.vector.tensor_scalar_mul(ln_m, S_col[:, 0:1], 1.0 / nt)
            l1 = small.tile([p, 1], f32, tag=tag + "l1", name=tag + "l1")
            nc.vector.tensor_mul(l1, S_col[:, 0:1], S_col[:, 0:1])
            nc.vector.tensor_scalar_mul(l1, l1, 1.0 / nt)
            nc.vector.tensor_sub(l1, S_col[:, 1:2], l1)
            ln_v = small.tile([p, 1], f32, tag=tag + "lv", name=tag + "lv")
            nc.vector.tensor_scalar_mul(ln_v, l1, 1.0 / nt1)
            inv_ln = quake_rsqrt(ln_v, p, tag + "il")

            rho = aff_sb[:, 0:1]
            t6 = small.tile([p, 1], f32, tag=tag + "t6", name=tag + "t6")
            nc.vector.tensor_mul(t6, rho, inv_ln)
            nc.vector.tensor_sub(t6, inv_ln, t6)
            A = small.tile([p, 1], f32, tag=tag + "A", name=tag + "A")
            nc.vector.tensor_add(A, t3, t6)
            u2 = small.tile([p, 1], f32, tag=tag + "u2", name=tag + "u2")
            nc.vector.tensor_mul(u2, ln_m, t6)
            nc.vector.tensor_add(u2, u1, u2)
            B = small.tile([p, 1], f32, tag=tag + "B", name=tag + "B")
            nc.vector.tensor_scalar_mul(B, u2, -1.0)
            # sb = (gamma*A/2, (gamma*B + beta)/2) -- halved for tanh path
            sb = small.tile([p, 2], f32, tag=tag + "sb", name=tag + "sb")
            nc.vector.tensor_mul(sb[:, 0:1], A, aff_sb[:, 1:2])
            nc.vector.tensor_scalar_mul(sb[:, 0:1], sb[:, 0:1], 0.5)
            nc.vector.tensor_mul(sb[:, 1:2], B, aff_sb[:, 1:2])
            nc.vector.tensor_add(sb[:, 1:2], sb[:, 1:2], aff_sb[:, 2:3])
            nc.vector.tensor_scalar_mul(sb[:, 1:2], sb[:, 1:2], 0.5)
            return sb

        def layer_sums_local(sums2, bounce_d, tag):
            """sums2 [128,2] -> layer [1,2] via a DRAM bounce + DVE free-dim
            reduce (no PSUM, no gpsimd)."""
            nc.sync.dma_start(out=bounce_d[:, :], in_=sums2)
            tr = small.tile([1, 2, 128], f32, tag=tag + "tr", name=tag + "tr")
            nc.sync.dma_start(
                out=tr, in_=bass.AP(tensor=bounce_d, offset=0,
                                    ap=[[0, 1], [1, 2], [2, 128]]))
            lsb = small.tile([1, 2], f32, tag=tag + "lsb", name=tag + "lsb")
            nc.vector.tensor_reduce(out=lsb.rearrange("p (a b) -> p a b", b=1),
                                    in_=tr, axis=mybir.AxisListType.X, op=ALU.add)
            return lsb

        def group_sum_bcast(tr, bounce_d, tag):
            """tr [1, 2, 4] f32 -> reduce over group axis, bounce, broadcast
            to [128, 2]."""
            tsb = small.tile([1, 2], f32, tag=tag + "tsb", name=tag + "tsb")
            nc.vector.tensor_reduce(out=tsb.rearrange("p (a b) -> p a b", b=1),
                                    in_=tr, axis=mybir.AxisListType.X, op=ALU.add)
            nc.sync.dma_start(out=bounce_d[:, :], in_=tsb)
            bc = small.tile([128, 2], f32, tag=tag + "bc", name=tag + "bc")
            nc.sync.dma_start(
                out=bc, in_=bass.AP(tensor=bounce_d, offset=0, ap=[[0, 128], [1, 2]]))
            return bc

        def emit_body():
            attn = ctx.enter_context(tc.tile_pool(name="attn", bufs=1))
            yrep_pool = ctx.enter_context(tc.tile_pool(name="yrep", bufs=1))

            # ---- big input loads issued first: spe alone on the gpsimd
            # queue (nothing may block ahead of the AG triggers later);
            # everything else on sync after ype.
            spe = yrep_pool.tile([128, 130, 130], bf16, tag="spe")
            for rb in range(5):
                r0, r1 = 26 * rb, 26 * rb + 26
                nc.gpsimd.dma_start(out=spe[:, r0:r1, :], in_=spepad_d[:, r0:r1, :])
            yrep = [yrep_pool.tile([128, 66, 66], bf16, tag=f"yrep{kt}",
                                   name=f"yrep{kt}") for kt in range(2)]
            sgate_sb = attn.tile([128, 4096], bf16, tag="sgate")

            # ================= PE warmup (HAM) =================
            warm_src = pers.tile([128, 512], bf16, tag="warmsrc")
            nc.vector.memset(warm_src[:, :], 0.0)
            with tc.tile_pool(name="warmps", bufs=1, space="PSUM") as wps_pool:
                wps = wps_pool.tile([128, 512], f32, tag="warmps")
                for w in range(9):
                    nc.tensor.matmul(wps[:, :], warm_src[:, 0:128], warm_src[:, :],
                                     start=True, stop=True, skip_group_check=True)

            # ================= q conv (tap-outer, 8 banks) =================
            qstats = small.tile([128, 8, 6], f32, tag="qstats")
            qrep = attn.tile([128, 4096], bf16, tag="qrep")
            with tc.tile_pool(name="inp", bufs=1) as inp, \
                 tc.tile_pool(name="qps", bufs=1, space="PSUM") as qps_pool:
                ype = [inp.tile([128, 66, 66], bf16, tag=f"ype{kt}", name=f"ype{kt}")
                       for kt in range(2)]
                for rb in range(3):
                    for kt in range(2):
                        r0, r1 = 22 * rb, 22 * rb + 22
                        eng = nc.sync if kt == 0 else nc.scalar
                        eng.dma_start(out=ype[kt][:, r0:r1, :], in_=ypepad_d[kt][:, r0:r1, :])
                nc.sync.dma_start(out=wg_sb, in_=wg_d[:, :, :].rearrange("t p m -> p t m"))
                nc.sync.dma_start(out=wp_sb, in_=wp_d[:, :, :].rearrange("t p m -> p t m"))
                for kt in range(2):
                    for rb in range(3):
                        r0, r1 = 22 * rb, 22 * rb + 22
                        nc.sync.dma_start(out=yrep[kt][:, r0:r1, :], in_=yreppad_d[kt][:, r0:r1, :])
                nc.sync.dma_start(out=sgate_sb, in_=sgate_d[:, :])

                # 4-way tap-col-tiled: (tap,kt) idx -> col strip idx%4; strips
                # hold partial sums, reduced on DVE afterwards.
                qps = qps_pool.tile([128, 4096], f32, tag="qps")
                strip_last = [16, 17, 14, 15]   # last idx landing in each strip
                for idx in range(18):
                    t, kt = idx // 2, idx % 2
                    ky, kx = t // 3, t % 3
                    st = idx % 4
                    for c in range(8):
                        r0 = 8 * c
                        nc.tensor.matmul(
                            qps[32 * st : 32 * st + 32, 512 * c : 512 * c + 512],
                            wq_sb[:, idx, :],
                            ype[kt][:, r0 + ky : r0 + ky + 8, kx : kx + 64],
                            start=(idx < 4), stop=(idx == strip_last[st]),
                            tile_position=(0, 32 * st),
                            skip_group_check=True,
                        )
                # strip sums -> qsum [32, 4096] bf16 + IN stats
                # (split across DVE and gpsimd to halve the serial chain)
                qsum = attn.tile([32, 4096], bf16, tag="qsum")
                qtmp = small.tile([32, 512], f32, tag="qtmp")
                qtmp2 = small.tile([32, 512], f32, tag="qtmp2")
                for c in range(8):
                    cols = slice(512 * c, 512 * c + 512)
                    t = qtmp if c % 2 == 0 else qtmp2
                    nc.vector.tensor_copy(t, qps[0:32, cols])
                    nc.vector.tensor_add(t, t, qps[32:64, cols])
                    nc.vector.tensor_add(t, t, qps[64:96, cols])
                    nc.vector.tensor_add(qsum[:, cols], t, qps[96:128, cols])
                    nc.vector.bn_stats(out=qstats[0:32, c, :], in_=qsum[:, cols])
                qmv = small.tile([32, 2], f32, tag="qmv")
                nc.vector.bn_aggr(out=qmv, in_=qstats[0:32])
                qinv = quake_rsqrt(qmv[:, 1:2], 32, "qinv")
                qinv2 = small.tile([32, 1], f32, tag="qinv2")
                nc.vector.tensor_scalar_mul(qinv2, qinv, 0.5)
                qbias2 = small.tile([32, 1], f32, tag="qbias2")
                nc.vector.tensor_scalar(out=qbias2, in0=qmv[:, 0:1], scalar1=qinv2,
                                        scalar2=-1.0, op0=ALU.mult, op1=ALU.mult)
                # silu via tanh: q = xq2 * (1 + tanh(xq2)), xq2 = xhat/2
                qth = attn.tile([32, 4096], bf16, tag="qth32")
                xq2 = attn.tile([32, 4096], bf16, tag="xq232")
                for hh in range(2):
                    cols = slice(2048 * hh, 2048 * hh + 2048)
                    nc.scalar.activation(out=qth[:, cols], in_=qsum[:, cols],
                                         func=AF.Tanh, bias=qbias2, scale=qinv2)
                    nc.vector.tensor_scalar(out=xq2[:, cols], in0=qsum[:, cols],
                                            scalar1=qinv2, scalar2=qbias2,
                                            op0=ALU.mult, op1=ALU.add)
                    nc.vector.scalar_tensor_tensor(
                        out=qrep[0:32, cols], in0=qth[:, cols], scalar=1.0,
                        in1=xq2[:, cols], op0=ALU.add, op1=ALU.mult)
                # replicate q to all four 32-row strips for row-tiled QK
                for rr in range(1, 4):
                    for hh in range(2):
                        cols = slice(2048 * hh, 2048 * hh + 2048)
                        nc.vector.tensor_copy(qrep[32 * rr : 32 * rr + 32, cols],
                                              qrep[0:32, cols])

            # ================= attention window =================
            greenraw = attn.tile([128, 4096], bf16, tag="greenraw")
            gstats = small.tile([128, 8, 6], f32, tag="gstats")
            vstats = small.tile([32, 8, 6], f32, tag="vstats")
            vraw = attn.tile([32, 4096], bf16, tag="vraw")
            mhapad = attn.tile([128, 66, 66], fp16, tag="mhapad")
            purpleraw = attn.tile([128, 4096], bf16, tag="purpleraw")
            pstats = small.tile([128, 8, 6], f32, tag="pstats")
            # v tap-outer output [128, 32, 48]: (j%128, jb, ch), ch padded to 48
            vtx = attn.tile([128, 32, 48], bf16, tag="vtx")
            vt8 = attn.tile([128, 32, 48], fp8, tag="vt8")
            vt8_pair = vt8.rearrange("p (pr ko) c -> p pr ko c", ko=2)
            gsums2 = None

            with tc.tile_pool(name="vtmp", bufs=2) as vtmp_pool, \
                 tc.tile_pool(name="stage", bufs=2) as stage_pool, \
                 tc.tile_pool(name="aexpp", bufs=12) as aexp_pool, \
                 tc.tile_pool(name="qkps", bufs=1, space="PSUM") as qkps, \
                 tc.tile_pool(name="pvps", bufs=2, space="PSUM") as pvps, \
                 tc.tile_pool(name="fill", bufs=2, space="PSUM") as fillps:
                expbias = small.tile([128, 1], f32, tag="expbias")
                nc.vector.memset(expbias, EXP_BIAS)
                ones_bf = small.tile([1, 32], bf16, tag="onesbf")
                nc.vector.memset(ones_bf, 1.0)

                # ---------- filler machinery ----------
                fillers = []

                def fill_step(n):
                    for _ in range(n):
                        if fillers:
                            fillers.pop(0)()

                # ---- v conv fillers (4-way tap-col-tiled, 3 rounds/chunk;
                # strips summed on DVE into vraw) ----
                V_GROUPS = [(0, 1), (2, 3), (4, 5), (6, 7)]
                V_STRIP_LAST = [8, 5, 6, 7]     # last tap landing in strip tap%4
                vsum_t = small.tile([32, 512], f32, tag="vsumt")

                def make_v_round(grp, gi, tiles_box, rnd):
                    def run():
                        if rnd == 0:
                            tiles_box.clear()
                            tiles_box.extend(
                                fillps.tile([128, 512], f32, tag="fill",
                                            name=f"vps{gi}_{c}") for c in grp)
                        for tap in range(4 * rnd, min(4 * rnd + 4, 9)):
                            ky, kx = tap // 3, tap % 3
                            st = tap % 4
                            for ci, c in enumerate(grp):
                                r0 = 8 * c
                                nc.tensor.matmul(
                                    tiles_box[ci][32 * st : 32 * st + 32, :],
                                    wv_sb[:, tap, :],
                                    spe[:, 2 * r0 + ky : 2 * r0 + ky + 16 : 2, kx : kx + 128 : 2],
                                    start=(tap < 4), stop=(tap == V_STRIP_LAST[st]),
                                    tile_position=(0, 32 * st),
                                    skip_group_check=True,
                                )
                        if rnd == 2:
                            for ci, c in enumerate(grp):
                                cols = slice(512 * c, 512 * c + 512)
                                t = tiles_box[ci]
                                nc.vector.tensor_copy(vsum_t, t[0:32, :])
                                nc.vector.tensor_add(vsum_t, vsum_t, t[32:64, :])
                                nc.vector.tensor_add(vsum_t, vsum_t, t[64:96, :])
                                nc.vector.tensor_add(vraw[:, cols], vsum_t, t[96:128, :])
                                nc.vector.bn_stats(out=vstats[:, c, :], in_=vraw[:, cols])
                    return run

                vbox = []
                for gi, grp in enumerate(V_GROUPS):
                    for rnd in range(3):
                        fillers.append(make_v_round(grp, gi, vbox, rnd))

                def v_finalize():
                    vmv = small.tile([32, 2], f32, tag="vmv")
                    nc.vector.bn_aggr(out=vmv, in_=vstats)
                    vinv = quake_rsqrt(vmv[:, 1:2], 32, "vinv")
                    vinv2 = small.tile([32, 1], f32, tag="vinv2")
                    nc.vector.tensor_scalar_mul(vinv2, vinv, 0.5)
                    vbias2 = small.tile([32, 1], f32, tag="vbias2")
                    nc.vector.tensor_scalar(out=vbias2, in0=vmv[:, 0:1], scalar1=vinv2,
                                            scalar2=-1.0, op0=ALU.mult, op1=ALU.mult)
                    xv2 = vtmp_pool.tile([32, 4096], bf16, tag="vtmp", name="xv2")
                    nc.vector.tensor_scalar(out=xv2, in0=vraw, scalar1=vinv2,
                                            scalar2=vbias2, op0=ALU.mult, op1=ALU.add)
                    vt32 = vtmp_pool.tile([32, 4096], bf16, tag="vtmp", name="vt32")
                    vt32_v = vt32.rearrange("p (m c) -> p m c", c=32)
                    for hh in range(2):
                        cols = slice(2048 * hh, 2048 * hh + 2048)
                        nc.vector.transpose(out=vt32[:, cols], in_=xv2[:, cols])
                        for w in range(4):
                            nc.gpsimd.dma_start(
                                out=vtx[32 * w : 32 * w + 32, 16 * hh : 16 * hh + 16, 0:32],
                                in_=vt32_v[:, 64 * hh + w : 64 * hh + 64 : 4, :])
                    # silu via tanh on the transposed data, cast to fp8
                    tv = vtmp_pool.tile([128, 32, 48], bf16, tag="vth", name="tv")
                    nc.scalar.activation(out=tv.rearrange("p a b -> p (a b)"),
                                         in_=vtx.rearrange("p a b -> p (a b)"),
                                         func=AF.Tanh)
                    with nc.allow_low_precision(reason="fp8 v for DoubleRow PV"):
                        nc.vector.scalar_tensor_tensor(
                            out=vt8.rearrange("p a b -> p (a b)"),
                            in0=tv.rearrange("p a b -> p (a b)"), scalar=1.0,
                            in1=vtx.rearrange("p a b -> p (a b)"),
                            op0=ALU.add, op1=ALU.mult)
                        nc.vector.memset(vt8[:, :, 32:33], 1.0)

                fillers.append(v_finalize)

                # ---- green conv fillers (phase-col-tiled: 8 rounds of 4
                # concurrent phase MMs per chunk; phase p owns partitions
                # 32p..32p+32 of the psum = its 32 output channels) ----
                G_GROUPS = [(0, 1), (2, 3), (4, 5), (6, 7)]

                def make_g_round(grp, gi, tiles_box, rnd):
                    # rnd = (dy*2+dx)*2 + kt in 0..7
                    def run():
                        if rnd == 0:
                            tiles_box.clear()
                            tiles_box.extend(
                                fillps.tile([128, 512], f32, tag="fill",
                                            name=f"gps{gi}_{c}") for c in grp)
                        dy, dx = rnd // 4, (rnd // 2) % 2
                        kt = rnd % 2
                        for p in range(4):
                            pr, pc = p // 2, p % 2
                            ey, ex = pr + dy, pc + dx
                            for ci, c in enumerate(grp):
                                r0 = 8 * c
                                nc.tensor.matmul(
                                    tiles_box[ci][32 * p : 32 * p + 32, :],
                                    wg_sb[:, 4 * rnd + p, :],
                                    yrep[kt][:, r0 + ey : r0 + ey + 8, ex : ex + 64],
                                    start=(rnd == 0), stop=(rnd == 7),
                                    tile_position=(0, 32 * p),
                                    skip_group_check=True,
                                )
                        if rnd == 7:
                            for ci, c in enumerate(grp):
                                cols = slice(512 * c, 512 * c + 512)
                                nc.vector.tensor_copy(greenraw[:, cols], tiles_box[ci][:, :])
                                nc.vector.bn_stats(out=gstats[:, c, :], in_=greenraw[:, cols])
                    return run

                gbox = []
                for gi, grp in enumerate(G_GROUPS):
                    for rnd in range(8):
                        fillers.append(make_g_round(grp, gi, gbox, rnd))

                # ---- purple conv fillers (phase-col-tiled, 4 rounds) ----
                def make_p_round(grp, gi, tiles_box, rnd):
                    # rnd = dy*2+dx in 0..3
                    def run():
                        if rnd == 0:
                            tiles_box.clear()
                            tiles_box.extend(
                                fillps.tile([128, 512], f32, tag="fill",
                                            name=f"pps{gi}_{c}") for c in grp)
                        dy, dx = rnd // 2, rnd % 2
                        for p in range(4):
                            pr, pc = p // 2, p % 2
                            ey, ex = pr + dy, pc + dx
                            for ci, c in enumerate(grp):
                                r0 = 8 * c
                                nc.tensor.matmul(
                                    tiles_box[ci][32 * p : 32 * p + 32, :],
                                    wp_sb[:, 4 * rnd + p, :],
                                    mhapad[:, r0 + ey : r0 + ey + 8, ex : ex + 64],
                                    start=(rnd == 0), stop=(rnd == 3),
                                    tile_position=(0, 32 * p),
                                    skip_group_check=True,
                                )
                        if rnd == 3:
                            for ci, c in enumerate(grp):
                                cols = slice(512 * c, 512 * c + 512)
                                nc.vector.tensor_copy(purpleraw[:, cols], tiles_box[ci][:, :])
                                nc.vector.bn_stats(out=pstats[:, c, :], in_=purpleraw[:, cols])
                    return run

                pbox = []

                def queue_purple(grp, gi):
                    for rnd in range(4):
                        fillers.append(make_p_round(grp, gi, pbox, rnd))

                def emit_divide(k):
                    """Pull AG chunk k's gathered mha rows into mhapad + edge pad.
                    Everything on the gpsimd queue: these ops gate on the
                    AllGather and must not block the DVE/PE/Sync queues."""
                    c0, c1 = AG_BOUNDS[k]
                    w = c1 - c0
                    r0, r1 = c0 // 64, c1 // 64       # image rows
                    nc.gpsimd.dma_start(
                        out=mhapad[:, r0 + 1 : r1 + 1, 1:65],
                        in_=bass.AP(tensor=ccout[k], offset=0,
                                    ap=[[33 * w, 4], [w, 32], [1, w]]))
                    nc.gpsimd.tensor_copy(
                        mhapad[:, r0 + 1 : r1 + 1, 0:1],
                        mhapad[:, r0 + 1 : r1 + 1, 1:2])
                    nc.gpsimd.tensor_copy(
                        mhapad[:, r0 + 1 : r1 + 1, 65:66],
                        mhapad[:, r0 + 1 : r1 + 1, 64:65])
                    if k == 0:
                        nc.gpsimd.tensor_copy(mhapad[:, 0:1, :], mhapad[:, 1:2, :])
                    if k == len(AG_BOUNDS) - 1:
                        nc.gpsimd.tensor_copy(mhapad[:, 65:66, :], mhapad[:, 64:65, :])

                # ---------- the attention I/g loop ----------
                ag_ends = {c1 // 512 - 1: k for k, (c0, c1) in enumerate(AG_BOUNDS)}
                pv_flush = []   # deferred PV work for I=0

                def emit_pv(pvt, g, aexp_t, first, last):
                    """Two fp8 DoubleRow MMs: pairs (2g, 2g+1) into pvt."""
                    for pi in range(2):
                        pair = 2 * g + pi
                        nc.tensor.matmul(
                            pvt[0:33, :],
                            vt8_pair[:, pair, :, 0:33],
                            aexp_t[:, pi, :, :],
                            start=(first and pi == 0), stop=(last and pi == 1),
                            perf_mode=DR,
                            skip_group_check=True,
                        )

                def emit_div_chain(I, pvt):
                    """Reciprocal + broadcast + stage + ccin DMA + AG send for
                    I-block I.  Deferred into the next I's units so the DVE/PE
                    queues never stall on it at an I boundary."""
                    rec = schraudolph_recip(
                        pvt[32:33, :], 512, stage_pool, f"rec{I}",
                        out_bf=stage_pool.tile([1, 512], bf16, tag="recbf",
                                               name=f"recbf{I}"))
                    rbs_ps = fillps.tile([128, 512], f32, tag="fill", name=f"rbsps{I}")
                    nc.tensor.matmul(rbs_ps[0:32, 0:512], ones_bf, rec,
                                     start=True, stop=True, skip_group_check=True)
                    rbs = stage_pool.tile([32, 512], f32, tag="rbs", name=f"rbs{I}")
                    nc.vector.tensor_copy(rbs, rbs_ps[0:32, 0:512])
                    stg = stage_pool.tile([32, 512], fp16, tag="stage", name=f"stg{I}")
                    with nc.allow_low_precision(reason="fp16 attention output"):
                        nc.vector.tensor_mul(stg, pvt[0:32, :], rbs)
                    sk = next(k for k, (c0, c1) in enumerate(AG_BOUNDS)
                              if c0 <= 512 * I < c1)
                    lc = 512 * I - AG_BOUNDS[sk][0]
                    nc.sync.dma_start(out=ccin[sk][0:32, lc : lc + 512], in_=stg)

                    if I == 5:
                        # green complete by now: in-band layer sums on chunk 2
                        gmv = small.tile([128, 2], f32, tag="gmv")
                        nc.vector.bn_aggr(out=gmv, in_=gstats)
                        nonlocal gsums2
                        gsums2 = part_sums(gmv, "gsums2")
                        glsb = layer_sums_local(gsums2, psum_b, "gl")
                        nc.sync.dma_start(out=ccin[AG_GREEN][32:33, 0:4],
                                          in_=glsb.bitcast(fp16))
                    if I in ag_ends:
                        k = ag_ends[I]
                        if no_cc:
                            for g4 in range(4):
                                nc.sync.dma_start(
                                    out=ccout[k][33 * g4 : 33 * g4 + 33, :],
                                    in_=ccin[k][:, :])
                        else:
                            nc.gpsimd.collective_compute(
                                "AllGather", ALU.bypass,
                                replica_groups=_REPLICA_GROUPS,
                                ins=[ccin[k][:, :]],
                                outs=[ccout[k][:, :]],
                            )

                pending_div = None
                for I in range(8):
                    pvt = pvps.tile([128, 512], f32, tag="pvt", name=f"pvt{I}")
                    for g in range(8):
                        if g == 2 and pending_div is not None:
                            emit_div_chain(*pending_div)
                            pending_div = None
                        # QK half-tiles pipeline against the other half's exp:
                        # QK r0/r1 wait only on exp_lo(g-1), r2/r3 on exp_hi(g-1).
                        qk4 = qkps.tile([128, 2048], f32, tag="qk4", name=f"qk4_{I}_{g}")
                        for r in range(4):
                            jb = 4 * g + r
                            nc.tensor.matmul(
                                qk4[:, 512 * r : 512 * r + 512],
                                qrep[32 * r : 32 * r + 32, 128 * jb : 128 * jb + 128],
                                qrep[32 * r : 32 * r + 32, 512 * I : 512 * I + 512],
                                start=True, stop=True,
                                tile_position=(32 * r, 0),
                                skip_group_check=True,
                            )
                        aexp = aexp_pool.tile([128, 2048], fp8, tag="aexp",
                                              name=f"ae{I}_{g}")
                        with nc.allow_low_precision(reason="fp8 attention weights"):
                            nc.scalar.activation(out=aexp[:, 0:1024], in_=qk4[:, 0:1024],
                                                 func=AF.Exp, scale=SCALE, bias=expbias)
                            nc.scalar.activation(out=aexp[:, 1024:2048], in_=qk4[:, 1024:2048],
                                                 func=AF.Exp, scale=SCALE, bias=expbias)
                        aexp_t = aexp.rearrange("p (pr ko i) -> p pr ko i", ko=2, i=512)
                        pv_flush.append((pvt, g, aexp_t, g == 0, g == 7))
                        if I > 0 or g == 7:
                            # steady state: emit PV one unit late so it never
                            # stalls the PE queue head on a fresh exp
                            while len(pv_flush) > (1 if g < 7 else 0):
                                args = pv_flush.pop(0)
                                emit_pv(args[0], args[1], args[2], args[3], args[4])
                        fill_step(1 if I > 0 else 2)

                    pending_div = (I, pvt)
                    # purple fillers as AG chunks land
                    if I == 5:
                        emit_divide(0)
                        queue_purple((0,), 0)
                    if I == 6:
                        emit_divide(1)
                        queue_purple((1, 2), 1)
                        queue_purple((3,), 2)

                emit_div_chain(*pending_div)
                pending_div = None

                emit_divide(2)
                queue_purple((4, 5), 3)
                fill_step(len(fillers))
                emit_divide(3)
                queue_purple((6,), 4)

                # ---- green finalize (overlaps the last AG) ----
                gx = small.tile([1, 16], fp16, tag="gx")
                w1 = AG_BOUNDS[AG_GREEN][1] - AG_BOUNDS[AG_GREEN][0]
                nc.sync.dma_start(
                    out=gx,
                    in_=bass.AP(tensor=ccout[AG_GREEN], offset=32 * w1,
                                ap=[[2, 2], [33 * w1, 4], [1, 2]]))
                glsum = group_sum_bcast(
                    gx.bitcast(f32).rearrange("p (k g) -> p k g", g=4), gls_b, "gl")
                gfill = fillps.tile([128, 512], f32, tag="fill", name="gch_ps")
                nc.tensor.matmul(gfill[:, 0:2], redmats_sb[:, 0:128], gsums2,
                                 start=True, stop=True, skip_group_check=True)
                gch = small.tile([128, 2], f32, tag="gach", name="gach")
                nc.vector.tensor_copy(gch, gfill[:, 0:2])
                gt3, gu1 = iln_local(gch, affg_sb, "ga")
                gsb = iln_post(gt3, gu1, glsum, affg_sb, "ga")
                # upy = silu(A*x+B) = u2*(1+tanh(u2)), u2 = x*A/2 + B/2
                # (reuse the q-phase slabs: those tiles are dead by now)
                gth = attn.tile([128, 4096], bf16, tag="qth32", name="gth")
                gu2 = attn.tile([128, 4096], bf16, tag="xq232", name="gu2")
                upy_sb = attn.tile([128, 4096], bf16, tag="qsum", name="upy")
                for hh in range(2):
                    cols = slice(2048 * hh, 2048 * hh + 2048)
                    nc.scalar.activation(out=gth[:, cols], in_=greenraw[:, cols],
                                         func=AF.Tanh, bias=gsb[:, 1:2], scale=gsb[:, 0:1])
                    nc.vector.tensor_scalar(out=gu2[:, cols], in0=greenraw[:, cols],
                                            scalar1=gsb[:, 0:1], scalar2=gsb[:, 1:2],
                                            op0=ALU.mult, op1=ALU.add)
                    nc.vector.scalar_tensor_tensor(
                        out=upy_sb[:, cols], in0=gth[:, cols], scalar=1.0,
                        in1=gu2[:, cols], op0=ALU.add, op1=ALU.mult)
                    nc.sync.dma_start(out=upyout_d[:, cols], in_=upy_sb[:, cols])

                emit_divide(4)
                queue_purple((7,), 5)
                fill_step(len(fillers))

                # ---- purple stats + AG + ILN ----
                pmv = small.tile([128, 2], f32, tag="pmv")
                nc.vector.bn_aggr(out=pmv, in_=pstats)
                psums2 = part_sums(pmv, "psums2")
                nc.sync.dma_start(out=psum_b2[:, :], in_=psums2)
                if no_cc:
                    for g4 in range(4):
                        nc.sync.dma_start(out=cp_out4[128 * g4 : 128 * g4 + 128, :],
                                          in_=psum_b2[:, :])
                else:
                    nc.gpsimd.collective_compute(
                        "AllGather", ALU.bypass,
                        replica_groups=_REPLICA_GROUPS,
                        ins=[bass.AP(tensor=psum_b2, offset=0, ap=[[2, 128], [1, 2]])],
                        outs=[bass.AP(tensor=cp_out4, offset=0, ap=[[2, 512], [1, 2]])],
                    )
                pfill = fillps.tile([128, 512], f32, tag="fill", name="pch_ps")
                nc.tensor.matmul(pfill[:, 0:2], redmats_sb[:, 0:128], psums2,
                                 start=True, stop=True, skip_group_check=True)
                pch = small.tile([128, 2], f32, tag="pach", name="pach")
                nc.vector.tensor_copy(pch, pfill[:, 0:2])
                pt3, pu1 = iln_local(pch, affp_sb, "pa")
                ptr = small.tile([128, 2, 4], f32, tag="ptr")
                nc.sync.dma_start(
                    out=ptr, in_=bass.AP(tensor=cp_out4, offset=0,
                                         ap=[[2, 128], [1, 2], [256, 4]]))
                pfill2 = fillps.tile([128, 512], f32, tag="fill", name="pls_ps")
                nc.tensor.matmul(pfill2[:, 0:8],
                                 redmats_sb[:, 128:256],
                                 ptr.rearrange("p a b -> p (a b)"),
                                 start=True, stop=True, skip_group_check=True)
                pls4 = small.tile([128, 2, 4], f32, tag="pls4", name="pls4")
                nc.vector.tensor_copy(pls4.rearrange("p a b -> p (a b)"), pfill2[:, 0:8])
                plsum = small.tile([128, 2], f32, tag="plsum")
                nc.vector.tensor_reduce(out=plsum.rearrange("p (a b) -> p a b", b=1),
                                        in_=pls4, axis=mybir.AxisListType.X, op=ALU.add)
                psb = iln_post(pt3, pu1, plsum, affp_sb, "pa")

                # z = sigmoid(A*x+B)*s = (1+tanh(u2))*(s/2), u2 = x*A/2+B/2
                pth = attn.tile([128, 4096], bf16, tag="qth32", name="pth")
                zpre = attn.tile([128, 4096], bf16, tag="xq232", name="zpre")
                for hh in range(4):
                    cols = slice(1024 * hh, 1024 * hh + 1024)
                    nc.scalar.activation(out=pth[:, cols], in_=purpleraw[:, cols],
                                         func=AF.Tanh, bias=psb[:, 1:2], scale=psb[:, 0:1])
                    nc.vector.scalar_tensor_tensor(
                        out=zpre[:, cols], in0=pth[:, cols], scalar=1.0,
                        in1=sgate_sb[:, cols], op0=ALU.add, op1=ALU.mult)
                    nc.sync.dma_start(out=zout_d[:, cols], in_=zpre[:, cols])

        emit_body()

    nc.compile()
    return nc


_NC_CACHE = None
RUN_KWARGS = {}      # test harness may set e.g. {"trace": True}
LAST_RESULTS = None  # BassKernelResults of the most recent run


def kernel(**inputs) -> np.ndarray:
    global _NC_CACHE, LAST_RESULTS
    from concourse.bass_utils import run_bass_kernel_spmd

    if _NC_CACHE is None:
        _NC_CACHE = build_bass()
    nc = _NC_CACHE

    in_maps = []
    for core in _CORES:
        ci = prepare_core_inputs(inputs, core)
        in_maps.append(ci)

    res = run_bass_kernel_spmd(nc, in_maps, _CORES, **RUN_KWARGS)
    LAST_RESULTS = res
    zs = [res.results[c]["zout"] for c in _CORES]
    upys = [res.results[c]["upyout"] for c in _CORES]
    return assemble_output(zs, upys)


if __name__ == "__main__":
    nc = build_bass()
    print("bass build OK")
